# revision 1
# baseline (speedup 1.0000x reference)
"""Trainium2 Bass kernel for nn_Decoder_80315888436037.

Two SPMD launches on 8 NeuronCores:
  A) attention+GRU recurrence, data-parallel over batch (4 batches/core)
  B) vocab projection (dec_h+0.5*sent) @ W_out + 1.5*b_out, vocab-sharded
     (4000 cols/core), fp32 data with f32r matmuls (tf32-like PE mode).
Host work between launches is only gather/reshape/transpose of activations.
"""

import numpy as np
import ml_dtypes

import concourse.bass as bass
import concourse.mybir as mybir
import concourse.tile as tile



import json

import concourse.bass_utils as _bu
import concourse.bass2jax as _b2j

_MAX_W = 1
_MAX_U = 1
_orig_compile_bir_kernel = _bu.compile_bir_kernel


def _split_sync(bir_json: bytes) -> bytes:
    m = json.loads(bir_json)
    uid = [0]

    def carrier(engine, debug, waits=None, updates=None):
        uid[0] += 1
        return {
            "debug": debug,
            "engine": engine,
            "ins": [],
            "name": f"WSPLIT-{uid[0]}",
            "opcode": "EventSemaphore",
            "outs": [],
            "sync_info": {"on_update": updates or [], "on_wait": waits or []},
        }

    changed = False
    for fn in m.get("functions", []):
        for bb in fn.get("blocks", []):
            out = []
            for inst in bb.get("instructions", []):
                si = inst.get("sync_info")
                if not si:
                    out.append(inst)
                    continue
                waits = si.get("on_wait") or []
                pre = []
                if len(waits) > _MAX_W:
                    changed = True
                    keep = waits[-_MAX_W:]
                    for w in waits[:-_MAX_W]:
                        pre.append(carrier(inst["engine"], inst.get("debug", 0), waits=[w]))
                    si["on_wait"] = keep
                out.extend(pre)
                out.append(inst)
            bb["instructions"] = out
    if not changed:
        return bir_json
    return json.dumps(m).encode()


def _patched_compile_bir_kernel(bir_json, tmpdir, neff_name="file.neff"):
    if isinstance(bir_json, str):
        bir_json = bir_json.encode()
    return _orig_compile_bir_kernel(_split_sync(bir_json), tmpdir, neff_name=neff_name)


def _apply_walrus_patch():
    _bu.compile_bir_kernel = _patched_compile_bir_kernel
    _b2j.compile_bir_kernel = _patched_compile_bir_kernel


# ---------------- recurrence (launch A) ----------------



F32R = mybir.dt.float32r
F32 = mybir.dt.float32
BF16 = mybir.dt.bfloat16
I32 = mybir.dt.int32
AF = mybir.ActivationFunctionType

T = 64
NB = 4  # batches per core


def build_rec(debug=False):
    nc = bass.Bass()
    hidT_d = nc.dram_tensor("hidT", [NB, 8, 128, 128], F32R, kind="ExternalInput")
    ws_d = nc.dram_tensor("ws", [4, 128, 1536], F32R, kind="ExternalInput")
    wc_d = nc.dram_tensor("wc", [8, 128, 1024], F32R, kind="ExternalInput")
    wenc_d = nc.dram_tensor("wenc", [8, 128, 512], F32R, kind="ExternalInput")
    wx_d = nc.dram_tensor("wx", [2, 128, 1024], F32R, kind="ExternalInput")
    bx_d = nc.dram_tensor("bx", [1, 1024], F32R, kind="ExternalInput")
    bebp_d = nc.dram_tensor("bebp", [128, 4], F32, kind="ExternalInput")
    watt_d = nc.dram_tensor("watt", [128, 4], BF16, kind="ExternalInput")
    s0_d = nc.dram_tensor("s0", [128, 16], F32R, kind="ExternalInput")
    sh_d = nc.dram_tensor("sh", [128, 16], F32, kind="ExternalInput")
    idx_d = nc.dram_tensor("idx", [128, 2], I32, kind="ExternalInput")
    eye_d = nc.dram_tensor("eye", [128, 128], F32, kind="ExternalInput")
    eyer_d = nc.dram_tensor("eyer", [128, 128], F32R, kind="ExternalInput")
    ones_d = nc.dram_tensor("onesr", [1, 128], F32R, kind="ExternalInput")
    z16_d = nc.dram_tensor("z16", [128, 16], F32R, kind="ExternalInput")
    embed_d = nc.dram_tensor("embed", [32000, 256], F32, kind="ExternalInput")
    dec_d = nc.dram_tensor("decT", [T, 128, 16], F32, kind="ExternalOutput")
    if debug:
        dbg_enc = nc.dram_tensor("dbg_enc", [128, 2048], F32, kind="ExternalOutput")
        dbg_pc = nc.dram_tensor("dbg_pc", [128, 4, 1024], F32, kind="ExternalOutput")
        dbg_xp = nc.dram_tensor("dbg_xp", [128, 2, 1024], F32, kind="ExternalOutput")
        dbg_emb = nc.dram_tensor("dbg_emb", [128, 2, 2, 128], F32, kind="ExternalOutput")
        dbg_q = nc.dram_tensor("dbg_q", [128, 16], F32, kind="ExternalOutput")
        dbg_sc = nc.dram_tensor("dbg_sc", [1, 512], F32, kind="ExternalOutput")
        dbg_ar = nc.dram_tensor("dbg_ar", [1, 512], F32, kind="ExternalOutput")
        dbg_atb = nc.dram_tensor("dbg_atb", [128, 16], F32, kind="ExternalOutput")
        dbg_gates = nc.dram_tensor("dbg_gates", [4, 1536], F32, kind="ExternalOutput")

    with tile.TileContext(nc) as tc:
        with (
            nc.allow_low_precision(reason="float32r tiles carry full fp32 bits"),
            tc.tile_pool(name="const", bufs=1) as cpool,
            tc.tile_pool(name="state", bufs=2) as spool,
            tc.tile_pool(name="work", bufs=2) as wpool,
            tc.tile_pool(name="stage", bufs=2) as stpool,
        ):
            # ---- resident constants/weights ----
            ws_t = cpool.tile([128, 4, 1536], F32R)
            wc_t = cpool.tile([128, 8, 1024], F32R)
            wenc_t = cpool.tile([128, 8, 512], F32R)
            wx_t = cpool.tile([128, 2, 1024], F32R)
            bx_t = cpool.tile([1, 1024], F32R)
            bebp_t = cpool.tile([128, 4], F32)
            watt_t = cpool.tile([128, 4], BF16)
            sh_t = cpool.tile([128, 16], F32)
            idx_t = cpool.tile([128, 2], I32)
            eye_t = cpool.tile([128, 128], F32)
            eyer_t = cpool.tile([128, 128], F32R)
            ones_t = cpool.tile([1, 128], F32R)
            for kc in range(4):
                nc.sync.dma_start(out=ws_t[:, kc, :], in_=ws_d[kc])
            for kc in range(8):
                nc.sync.dma_start(out=wc_t[:, kc, :], in_=wc_d[kc])
                nc.sync.dma_start(out=wenc_t[:, kc, :], in_=wenc_d[kc])
            for kc in range(2):
                nc.sync.dma_start(out=wx_t[:, kc, :], in_=wx_d[kc])
            nc.sync.dma_start(out=bx_t[:], in_=bx_d[:])
            nc.sync.dma_start(out=bebp_t[:], in_=bebp_d[:])
            nc.sync.dma_start(out=watt_t[:], in_=watt_d[:])
            nc.sync.dma_start(out=sh_t[:], in_=sh_d[:])
            nc.sync.dma_start(out=idx_t[:], in_=idx_d[:])
            nc.sync.dma_start(out=eye_t[:], in_=eye_d[:])
            nc.sync.dma_start(out=eyer_t[:], in_=eyer_d[:])
            nc.sync.dma_start(out=ones_t[:], in_=ones_d[:])

            encT_t = cpool.tile([128, 2048], F32)     # (b, hc, s)
            pc_t = cpool.tile([128, 4, 1024], F32R)   # [s, b, n]
            embT_t = cpool.tile([128, 2, 2, 128], F32R)
            xp_t = cpool.tile([128, 2, 1024], F32R)

            # ---- prologue: gather, X_pack, encT, P_c ----
            with (
                tc.tile_pool(name="pro", bufs=2) as propool,
                tc.tile_pool(name="props", bufs=1, space="PSUM") as propspool,
            ):
                for th in range(2):
                    erows = propool.tile([128, 256], F32, tag="erows")
                    nc.gpsimd.indirect_dma_start(
                        out=erows[:],
                        out_offset=None,
                        in_=embed_d[:],
                        in_offset=bass.IndirectOffsetOnAxis(
                            ap=idx_t[:, th:th + 1], axis=0),
                    )
                    for kc in range(2):
                        ptr = propspool.tile([128, 128], F32, tag="ptr")
                        nc.tensor.transpose(
                            out=ptr[:], in_=erows[:, kc * 128:(kc + 1) * 128],
                            identity=eye_t[:],
                        )
                        nc.vector.tensor_copy(out=embT_t[:, kc, th, :], in_=ptr[:])

                for th in range(2):
                    psx = propspool.tile([128, 1024], F32, tag="psx")
                    for n2 in range(2):
                        sl = slice(n2 * 512, (n2 + 1) * 512)
                        for kc in range(2):
                            nc.tensor.matmul(
                                psx[:, sl], embT_t[:, kc, th, :], wx_t[:, kc, sl],
                                start=(kc == 0), stop=False,
                            )
                        nc.tensor.matmul(
                            psx[:, sl], ones_t[:], bx_t[:, sl],
                            start=False, stop=True,
                        )
                    nc.vector.tensor_copy(out=xp_t[:, th, :], in_=psx[:])

                for b in range(NB):
                    hb = propool.tile([128, 8, 128], F32R, tag="hb")
                    for kc in range(8):
                        nc.sync.dma_start(out=hb[:, kc, :], in_=hidT_d[b, kc])
                    for hc in range(4):
                        pse = propspool.tile([128, 128], F32, tag="pse")
                        for kc in range(8):
                            nc.tensor.matmul(
                                pse[:], wenc_t[:, kc, hc * 128:(hc + 1) * 128],
                                hb[:, kc, :],
                                start=(kc == 0), stop=(kc == 7),
                            )
                        nc.vector.tensor_scalar_add(
                            encT_t[:, b * 512 + hc * 128: b * 512 + (hc + 1) * 128],
                            pse[:], bebp_t[:, hc:hc + 1],
                        )
                    psp = propspool.tile([128, 1024], F32, tag="psp")
                    for n2 in range(2):
                        sl = slice(n2 * 512, (n2 + 1) * 512)
                        for kc in range(8):
                            nc.tensor.matmul(
                                psp[:, sl], hb[:, kc, :], wc_t[:, kc, sl],
                                start=(kc == 0), stop=(kc == 7),
                            )
                    nc.vector.tensor_copy(out=pc_t[:, b, :], in_=psp[:])

            # ---- state ----
            sT = spool.tile([128, 16], F32R, tag="sT")
            nc.sync.dma_start(out=sT[:], in_=s0_d[:])
            atb = cpool.tile([128, 16], F32R)  # block-diag alphaT: col 5b = alpha_b
            nc.sync.dma_start(out=atb[:], in_=z16_d[:])
            if debug:
                nc.sync.dma_start(out=dbg_enc[:], in_=encT_t[:])
                nc.gpsimd.dma_start(out=dbg_pc[:], in_=pc_t[:])
                nc.gpsimd.dma_start(out=dbg_xp[:], in_=xp_t[:])
                nc.gpsimd.dma_start(out=dbg_emb[:], in_=embT_t[:])

            # ---- recurrence ----
            with (
                tc.tile_pool(name="psA", bufs=3, space="PSUM") as psA,
                tc.tile_pool(name="psB", bufs=2, space="PSUM") as psB,
            ):
                stg = None
                for t in range(T):
                    tm, th = t % 32, t // 32
                    # 1. q matmuls
                    psc = psA.tile([128, 1024], F32, tag="psc")
                    psq = psB.tile([128, 16], F32, tag="misc")
                    for hc in range(4):
                        for kc in range(4):
                            nc.tensor.matmul(
                                psq[:, 4 * hc:4 * hc + 4],
                                ws_t[:, kc, hc * 128:(hc + 1) * 128],
                                sT[:, 4 * kc:4 * kc + 4],
                                start=(kc == 0), stop=(kc == 3),
                                skip_group_check=True,
                            )
                    qT = wpool.tile([128, 16], F32, tag="qT")
                    nc.vector.tensor_copy(out=qT[:], in_=psq[:])
                    if debug and t == 0:
                        nc.sync.dma_start(out=dbg_q[:], in_=qT[:])
                    # 2. hT = tanh(encT + qT)
                    hT = wpool.tile([128, 2048], BF16, tag="hT")
                    hpre = wpool.tile([128, 2048], F32, tag="hpre")
                    for b in range(NB):
                        for hc in range(4):
                            sl = slice(b * 512 + hc * 128, b * 512 + (hc + 1) * 128)
                            nc.vector.tensor_scalar_add(
                                hpre[:, sl], encT_t[:, sl],
                                qT[:, 4 * hc + b:4 * hc + b + 1],
                            )
                        nc.scalar.activation(
                            hT[:, b * 512:(b + 1) * 512],
                            hpre[:, b * 512:(b + 1) * 512], AF.Tanh,
                        )
                    # 3. score + alpha
                    pssc = psB.tile([1, 512], F32, tag="misc")
                    for b in range(NB):
                        for hc in range(4):
                            nc.tensor.matmul(
                                pssc[0:1, b * 128:(b + 1) * 128],
                                watt_t[:, hc:hc + 1],
                                hT[:, b * 512 + hc * 128: b * 512 + (hc + 1) * 128],
                                start=(hc == 0), stop=(hc == 3),
                                skip_group_check=True,
                            )
                    arow = wpool.tile([1, 512], F32, tag="arow")
                    sums = wpool.tile([1, 4], F32, tag="sums")
                    for b in range(NB):
                        nc.scalar.activation(
                            arow[0:1, b * 128:(b + 1) * 128],
                            pssc[0:1, b * 128:(b + 1) * 128],
                            AF.Exp, accum_out=sums[0:1, b:b + 1],
                        )
                    if debug and t == 0:
                        scrow = wpool.tile([1, 512], F32, tag="scrow")
                        nc.vector.tensor_copy(out=scrow[:], in_=pssc[:])
                        nc.sync.dma_start(out=dbg_sc[:], in_=scrow[:])
                        nc.sync.dma_start(out=dbg_ar[:], in_=arow[:])
                    rsum = wpool.tile([1, 4], F32R, tag="rsum")
                    nc.vector.reciprocal(out=rsum[:], in_=sums[:])
                    psrs = psB.tile([128, 4], F32, tag="misc")
                    nc.tensor.matmul(psrs[:], ones_t[:], rsum[:])
                    rsb = wpool.tile([128, 4], F32, tag="rsb")
                    nc.vector.tensor_copy(out=rsb[:], in_=psrs[:])
                    psa = psB.tile([128, 4], F32, tag="misc")
                    for b in range(NB):
                        nc.tensor.transpose(
                            out=psa[:, b:b + 1],
                            in_=arow[0:1, b * 128:(b + 1) * 128],
                            identity=eye_t[0:1, 0:1])
                    for b in range(NB):
                        nc.vector.tensor_mul(
                            atb[:, 5 * b:5 * b + 1], psa[:, b:b + 1],
                            rsb[:, b:b + 1])
                    if debug and t == 0:
                        nc.gpsimd.dma_start(out=dbg_atb[:], in_=atb[:])
                    # 4. r/st matmuls
                    for g in (1, 2):
                        o = psc[0:4, (g - 1) * 512:g * 512]
                        xsl = slice((g - 1) * 512, g * 512)
                        for kc in range(4):
                            nc.tensor.matmul(
                                o, sT[:, 4 * kc:4 * kc + 4],
                                ws_t[:, kc, g * 512:(g + 1) * 512],
                                start=(kc == 0), stop=False,
                                skip_group_check=True,
                            )
                        nc.tensor.matmul(
                            o, eyer_t[:, 4 * tm:4 * tm + 4], xp_t[:, th, xsl],
                            start=False, stop=False,
                            skip_group_check=True,
                        )
                    for g in (1, 2):
                        o = psc[0:4, (g - 1) * 512:g * 512]
                        xsl = slice((g - 1) * 512, g * 512)
                        for bp in range(NB):
                            nc.tensor.matmul(
                                o, atb[:, 4 * bp:4 * bp + 4], pc_t[:, bp, xsl],
                                start=False, stop=(bp == NB - 1),
                                skip_group_check=True,
                            )
                    # 5. gates
                    trows = wpool.tile([4, 512], F32, tag="trows")
                    strows = wpool.tile([4, 512], F32, tag="strows")
                    nc.scalar.activation(trows[:], psc[0:4, 0:512], AF.Tanh, scale=0.5)
                    nc.scalar.activation(strows[:], psc[0:4, 512:1024], AF.Tanh)
                    psg = psB.tile([128, 32], F32, tag="misc")
                    for hc in range(4):
                        nc.tensor.transpose(
                            out=psg[:, 4 * hc:4 * hc + 4],
                            in_=trows[:, hc * 128:(hc + 1) * 128],
                            identity=eye_t[0:4, 0:4],
                        )
                        nc.tensor.transpose(
                            out=psg[:, 16 + 4 * hc:16 + 4 * hc + 4],
                            in_=strows[:, hc * 128:(hc + 1) * 128],
                            identity=eye_t[0:4, 0:4],
                        )
                    # 6. update: r = 0.5*tau + 0.5 ; s' = s + r*(st - s)
                    r_t = wpool.tile([128, 16], F32, tag="r_t")
                    nc.vector.tensor_scalar(
                        r_t[:], psg[:, 0:16], 0.5, 0.5,
                        mybir.AluOpType.mult, mybir.AluOpType.add)
                    d_t = wpool.tile([128, 16], F32, tag="d_t")
                    nc.vector.tensor_sub(d_t[:], psg[:, 16:32], sT[:])
                    p_t = wpool.tile([128, 16], F32, tag="p_t")
                    nc.vector.tensor_mul(p_t[:], r_t[:], d_t[:])
                    sTn = spool.tile([128, 16], F32R, tag="sT")
                    nc.vector.tensor_add(sTn[:], sT[:], p_t[:])
                    sT = sTn
                    # stage output (+0.5*sentiment), DMA every 8 steps
                    if t % 8 == 0:
                        stg = stpool.tile([128, 8, 16], F32, tag="stg")
                    nc.vector.tensor_add(stg[:, t % 8, :], sT[:], sh_t[:])
                    if t % 8 == 7:
                        nc.sync.dma_start(
                            out=dec_d[t - 7:t + 1].rearrange("t p j -> p t j"),
                            in_=stg[:])
    return nc


def host_prep(content, sentiment, hiddens, target, embed,
              W_enc, b_enc, W_prev, b_prev, W_att,
              Wi_g, bi_g, Wh_g, bh_g, Wc_g, bc_g,
              Wi, bi, Wh, bh, Wc, bc, core):
    """Build the per-core input map (batches 4*core .. 4*core+3)."""
    import numpy as np
    import ml_dtypes
    bs = slice(4 * core, 4 * core + 4)
    hid = hiddens[bs]                                    # [4,128,1024]
    hidT = np.ascontiguousarray(hid.transpose(0, 2, 1).reshape(4, 8, 128, 128))
    ws = np.concatenate([W_prev, Wh_g[:, :512], Wh], axis=1)      # [512,1536]
    ws = np.ascontiguousarray(ws.reshape(4, 128, 1536))
    wcm = np.concatenate([Wc_g[:, :512], Wc], axis=1)             # [1024,1024]
    wcm = np.ascontiguousarray(wcm.reshape(8, 128, 1024))
    wenc = np.ascontiguousarray(W_enc.reshape(8, 128, 512))
    wx = np.concatenate([Wi_g[:, :512], Wi], axis=1)              # [256,1024]
    wx = np.ascontiguousarray(wx.reshape(2, 128, 1024))
    bx = (np.concatenate([bi_g[:512] + bh_g[:512] + bc_g[:512], bi + bh + bc])
          .reshape(1, 1024))
    bebp = np.ascontiguousarray((b_enc + b_prev).reshape(4, 128).T)
    watt = np.ascontiguousarray(
        W_att[:, 0].reshape(4, 128).T).astype(ml_dtypes.bfloat16)
    s0 = np.zeros((128, 16), np.float32)
    sh = np.zeros((128, 16), np.float32)
    for b in range(4):
        for kc in range(4):
            s0[:, 4 * kc + b] = content[4 * core + b, kc * 128:(kc + 1) * 128]
            sh[:, 4 * kc + b] = 0.5 * sentiment[4 * core + b, kc * 128:(kc + 1) * 128]
    idx = np.zeros((128, 2), np.int32)
    for th in range(2):
        for tm in range(32):
            for b in range(4):
                idx[tm * 4 + b, th] = target[4 * core + b, th * 32 + tm]
    eye = np.eye(128, dtype=np.float32)
    return dict(
        hidT=hidT.astype(np.float32), ws=ws.astype(np.float32),
        wc=wcm.astype(np.float32), wenc=wenc.astype(np.float32),
        wx=wx.astype(np.float32), bx=bx.astype(np.float32),
        bebp=bebp.astype(np.float32), watt=watt,
        s0=s0, sh=sh, idx=idx, eye=eye, eyer=eye.copy(),
        onesr=np.ones((1, 128), np.float32), z16=np.zeros((128, 16), np.float32),
        embed=embed.astype(np.float32),
    )


def dec_from_out(decT):
    """decT [64,128,16] -> A_local [4, 64, 512] (dec_h + 0.5*sent)."""
    import numpy as np
    out = np.zeros((4, 64, 512), np.float32)
    for b in range(4):
        for kc in range(4):
            out[b, :, kc * 128:(kc + 1) * 128] = decT[:, :, 4 * kc + b]
    return out


# ---------------- projection (launch B) ----------------



KC = 4          # contraction chunks (H=512)
MV = 32         # vocab 128-blocks per shard (4096 padded)
NT = 4          # BT=2048 -> 4 chunks of 512
BT = 2048
F32R = mybir.dt.float32r
F32 = mybir.dt.float32


def build_proj():
    nc = bass.Bass()
    at_d = nc.dram_tensor("at", [KC, 128, BT], F32R, kind="ExternalInput")
    wo_d = nc.dram_tensor("wo", [KC, 128, MV * 128], F32R, kind="ExternalInput")
    bo_d = nc.dram_tensor("bo", [128, MV], F32, kind="ExternalInput")
    out_d = nc.dram_tensor("outT", [MV * 128, BT], F32, kind="ExternalOutput")
    outv = out_d.rearrange("(m p) n -> m p n", p=128)

    with tile.TileContext(nc) as tc:
        with (
            tc.tile_pool(name="weights", bufs=1) as wpool,
            tc.tile_pool(name="outs", bufs=6) as opool,
            tc.tile_pool(name="psum", bufs=8, space="PSUM") as ppool,
        ):
            at_t = wpool.tile([128, KC, BT], F32R)
            wo_t = wpool.tile([128, KC, MV * 128], F32R)
            bo_t = wpool.tile([128, MV], F32)
            nc.sync.dma_start(out=bo_t[:], in_=bo_d[:])
            for kc in range(KC):
                for nt in range(NT):
                    sl = slice(nt * 512, (nt + 1) * 512)
                    nc.sync.dma_start(out=at_t[:, kc, sl], in_=at_d[kc, :, sl])
                for mg in range(8):
                    sl = slice(mg * 512, (mg + 1) * 512)
                    nc.sync.dma_start(out=wo_t[:, kc, sl], in_=wo_d[kc, :, sl])

            for m in range(MV):
                for nt in range(NT):
                    ps = ppool.tile([128, 512], F32)
                    for kc in range(KC):
                        nc.tensor.matmul(
                            ps[:],
                            wo_t[:, kc, m * 128:(m + 1) * 128],
                            at_t[:, kc, nt * 512:(nt + 1) * 512],
                            start=(kc == 0),
                            stop=(kc == KC - 1),
                        )
                    ot = opool.tile([128, 512], F32)
                    nc.vector.tensor_scalar_add(ot[:], ps[:], bo_t[:, m:m + 1])
                    nc.sync.dma_start(
                        out=outv[m, :, nt * 512:(nt + 1) * 512], in_=ot[:]
                    )
    return nc




# ---------------- orchestration ----------------

_B, _T, _H, _V = 32, 64, 512, 32000
_VS = _V // 8          # vocab shard
_VP = 4096             # padded shard

_cache = {}
LAST_PERF = {}


def _trace_flag():
    import os
    return bool(int(os.environ.get("BASS_KERNEL_TRACE", "0")))


def _get_progs():
    if "rec" not in _cache:
        _apply_walrus_patch()
        _cache["rec"] = build_rec()
        _cache["proj"] = build_proj()
    return _cache["rec"], _cache["proj"]


def kernel(content, sentiment, hiddens, target, embed,
           W_enc, b_enc, W_prev, b_prev, W_att, b_att,
           Wi_g, bi_g, Wh_g, bh_g, Wc_g, bc_g,
           Wi, bi, Wh, bh, Wc, bc, W_out, b_out):
    from concourse.bass_utils import run_bass_kernel_spmd

    content = np.asarray(content, np.float32)
    sentiment = np.asarray(sentiment, np.float32)
    hiddens = np.asarray(hiddens, np.float32)
    target = np.asarray(target, np.int32)
    embed = np.asarray(embed, np.float32)

    rec_nc, proj_nc = _get_progs()
    trace = _trace_flag()

    in_maps_a = [
        host_prep(content, sentiment, hiddens, target, embed,
                  np.asarray(W_enc, np.float32), np.asarray(b_enc, np.float32),
                  np.asarray(W_prev, np.float32), np.asarray(b_prev, np.float32),
                  np.asarray(W_att, np.float32),
                  np.asarray(Wi_g, np.float32), np.asarray(bi_g, np.float32),
                  np.asarray(Wh_g, np.float32), np.asarray(bh_g, np.float32),
                  np.asarray(Wc_g, np.float32), np.asarray(bc_g, np.float32),
                  np.asarray(Wi, np.float32), np.asarray(bi, np.float32),
                  np.asarray(Wh, np.float32), np.asarray(bh, np.float32),
                  np.asarray(Wc, np.float32), np.asarray(bc, np.float32), core)
        for core in range(8)
    ]
    res_a = run_bass_kernel_spmd(rec_nc, in_maps_a, core_ids=list(range(8)),
                                 trace=trace)
    A = np.empty((_B, _T, _H), np.float32)
    for core in range(8):
        A[4 * core:4 * core + 4] = dec_from_out(res_a.results[core]["decT"])

    A2 = A.reshape(_B * _T, _H)
    at = np.ascontiguousarray(A2.T.reshape(4, 128, _B * _T))
    W_out = np.asarray(W_out, np.float32)
    b_out = np.asarray(b_out, np.float32)
    in_maps_b = []
    for core in range(8):
        wsh = np.zeros((_H, _VP), np.float32)
        wsh[:, :_VS] = W_out[:, core * _VS:(core + 1) * _VS]
        bsh = np.zeros(_VP, np.float32)
        bsh[:_VS] = 1.5 * b_out[core * _VS:(core + 1) * _VS]
        in_maps_b.append(dict(
            at=at,
            wo=np.ascontiguousarray(wsh.reshape(4, 128, _VP)),
            bo=np.ascontiguousarray(bsh.reshape(_VP // 128, 128).T),
        ))
    res_b = run_bass_kernel_spmd(proj_nc, in_maps_b, core_ids=list(range(8)),
                                 trace=trace)
    out = np.empty((_B, _T, _V), np.float32)
    for core in range(8):
        sh = res_b.results[core]["outT"][:_VS]          # [4000, 2048]
        out[:, :, core * _VS:(core + 1) * _VS] = (
            sh.T.reshape(_B, _T, _VS))

    if trace:
        LAST_PERF["rec_ns"] = res_a.exec_time_ns
        LAST_PERF["proj_ns"] = res_b.exec_time_ns
    return out



# revision 11
# speedup vs baseline: 1.0422x; 1.0422x over previous
"""Trainium2 Bass kernel for nn_Decoder_80315888436037.

Two SPMD launches on 8 NeuronCores:
  A) attention+GRU recurrence, data-parallel over batch (4 batches/core)
  B) vocab projection (dec_h+0.5*sent) @ W_out + 1.5*b_out, vocab-sharded
     (4000 cols/core), fp32 data with f32r matmuls (tf32-like PE mode).
Host work between launches is only gather/reshape/transpose of activations.
"""

import numpy as np
import ml_dtypes

import concourse.bass as bass
import concourse.mybir as mybir
import concourse.tile as tile



import json

import concourse.bass_utils as _bu
import concourse.bass2jax as _b2j

_MAX_W = 1
_MAX_U = 1
_orig_compile_bir_kernel = _bu.compile_bir_kernel


def _split_sync(bir_json: bytes) -> bytes:
    m = json.loads(bir_json)
    uid = [0]

    def carrier(engine, debug, waits=None, updates=None):
        uid[0] += 1
        return {
            "debug": debug,
            "engine": engine,
            "ins": [],
            "name": f"WSPLIT-{uid[0]}",
            "opcode": "EventSemaphore",
            "outs": [],
            "sync_info": {"on_update": updates or [], "on_wait": waits or []},
        }

    changed = False
    for fn in m.get("functions", []):
        for bb in fn.get("blocks", []):
            out = []
            for inst in bb.get("instructions", []):
                si = inst.get("sync_info")
                if not si:
                    out.append(inst)
                    continue
                waits = si.get("on_wait") or []
                pre = []
                if len(waits) > _MAX_W:
                    changed = True
                    keep = waits[-_MAX_W:]
                    for w in waits[:-_MAX_W]:
                        pre.append(carrier(inst["engine"], inst.get("debug", 0), waits=[w]))
                    si["on_wait"] = keep
                out.extend(pre)
                out.append(inst)
            bb["instructions"] = out
    if not changed:
        return bir_json
    return json.dumps(m).encode()


def _patched_compile_bir_kernel(bir_json, tmpdir, neff_name="file.neff"):
    if isinstance(bir_json, str):
        bir_json = bir_json.encode()
    return _orig_compile_bir_kernel(_split_sync(bir_json), tmpdir, neff_name=neff_name)


def _apply_walrus_patch():
    _bu.compile_bir_kernel = _patched_compile_bir_kernel
    _b2j.compile_bir_kernel = _patched_compile_bir_kernel


# ---------------- recurrence (launch A) ----------------



F32R = mybir.dt.float32r
F32 = mybir.dt.float32
BF16 = mybir.dt.bfloat16
I32 = mybir.dt.int32
AF = mybir.ActivationFunctionType

T = 64
NB = 4  # batches per core


def build_rec(debug=False):
    nc = bass.Bass()
    hidT_d = nc.dram_tensor("hidT", [NB, 8, 128, 128], F32R, kind="ExternalInput")
    ws_d = nc.dram_tensor("ws", [4, 128, 1536], F32R, kind="ExternalInput")
    wc_d = nc.dram_tensor("wc", [8, 128, 1024], F32R, kind="ExternalInput")
    wenc_d = nc.dram_tensor("wenc", [8, 128, 512], F32R, kind="ExternalInput")
    wx_d = nc.dram_tensor("wx", [2, 128, 1024], F32R, kind="ExternalInput")
    bx_d = nc.dram_tensor("bx", [1, 1024], F32R, kind="ExternalInput")
    bebp_d = nc.dram_tensor("bebp", [128, 4], F32, kind="ExternalInput")
    watt_d = nc.dram_tensor("watt", [128, 4], BF16, kind="ExternalInput")
    s0_d = nc.dram_tensor("s0", [128, 16], F32R, kind="ExternalInput")
    sh_d = nc.dram_tensor("sh", [128, 16], F32, kind="ExternalInput")
    idx_d = nc.dram_tensor("idx", [128, 2], I32, kind="ExternalInput")
    eye_d = nc.dram_tensor("eye", [128, 128], F32, kind="ExternalInput")
    eyer_d = nc.dram_tensor("eyer", [128, 128], F32R, kind="ExternalInput")
    ones_d = nc.dram_tensor("onesr", [1, 128], F32R, kind="ExternalInput")
    z16_d = nc.dram_tensor("z16", [128, 16], F32R, kind="ExternalInput")
    embed_d = nc.dram_tensor("embed", [32000, 256], F32, kind="ExternalInput")
    dec_d = nc.dram_tensor("decT", [T, 128, 16], F32, kind="ExternalOutput")
    if debug:
        dbg_enc = nc.dram_tensor("dbg_enc", [128, 2048], F32, kind="ExternalOutput")
        dbg_pc = nc.dram_tensor("dbg_pc", [128, 4, 1024], F32, kind="ExternalOutput")
        dbg_xp = nc.dram_tensor("dbg_xp", [128, 2, 1024], F32, kind="ExternalOutput")
        dbg_emb = nc.dram_tensor("dbg_emb", [128, 2, 2, 128], F32, kind="ExternalOutput")
        dbg_q = nc.dram_tensor("dbg_q", [128, 16], F32, kind="ExternalOutput")
        dbg_sc = nc.dram_tensor("dbg_sc", [1, 512], F32, kind="ExternalOutput")
        dbg_ar = nc.dram_tensor("dbg_ar", [1, 512], F32, kind="ExternalOutput")
        dbg_atb = nc.dram_tensor("dbg_atb", [128, 16], F32, kind="ExternalOutput")
        dbg_gates = nc.dram_tensor("dbg_gates", [4, 1536], F32, kind="ExternalOutput")

    with tile.TileContext(nc) as tc:
        with (
            nc.allow_low_precision(reason="float32r tiles carry full fp32 bits"),
            tc.tile_pool(name="const", bufs=1) as cpool,
            tc.tile_pool(name="state", bufs=2) as spool,
            tc.tile_pool(name="work", bufs=2) as wpool,
            tc.tile_pool(name="stage", bufs=2) as stpool,
        ):
            # ---- resident constants/weights ----
            ws_t = cpool.tile([128, 4, 1536], F32R)
            wc_t = cpool.tile([128, 8, 1024], F32R)
            wenc_t = cpool.tile([128, 8, 512], F32R)
            wx_t = cpool.tile([128, 2, 1024], F32R)
            bx_t = cpool.tile([1, 1024], F32R)
            bebp_t = cpool.tile([128, 4], F32)
            watt_t = cpool.tile([128, 4], BF16)
            sh_t = cpool.tile([128, 16], F32)
            idx_t = cpool.tile([128, 2], I32)
            eye_t = cpool.tile([128, 128], F32)
            eyer_t = cpool.tile([128, 128], F32R)
            ones_t = cpool.tile([1, 128], F32R)
            for kc in range(4):
                nc.sync.dma_start(out=ws_t[:, kc, :], in_=ws_d[kc])
            for kc in range(8):
                nc.sync.dma_start(out=wc_t[:, kc, :], in_=wc_d[kc])
                nc.sync.dma_start(out=wenc_t[:, kc, :], in_=wenc_d[kc])
            for kc in range(2):
                nc.sync.dma_start(out=wx_t[:, kc, :], in_=wx_d[kc])
            nc.sync.dma_start(out=bx_t[:], in_=bx_d[:])
            nc.sync.dma_start(out=bebp_t[:], in_=bebp_d[:])
            nc.sync.dma_start(out=watt_t[:], in_=watt_d[:])
            nc.sync.dma_start(out=sh_t[:], in_=sh_d[:])
            nc.sync.dma_start(out=idx_t[:], in_=idx_d[:])
            nc.sync.dma_start(out=eye_t[:], in_=eye_d[:])
            nc.sync.dma_start(out=eyer_t[:], in_=eyer_d[:])
            nc.sync.dma_start(out=ones_t[:], in_=ones_d[:])

            encT_t = cpool.tile([128, 2048], F32)     # (b, hc, s)
            pc_t = cpool.tile([128, 4, 1024], F32R)   # [s, b, n]
            embT_t = cpool.tile([128, 2, 2, 128], F32R)
            xp_t = cpool.tile([128, 2, 1024], F32R)

            # ---- prologue: gather, X_pack, encT, P_c ----
            with (
                tc.tile_pool(name="pro", bufs=2) as propool,
                tc.tile_pool(name="props", bufs=1, space="PSUM") as propspool,
            ):
                for th in range(2):
                    erows = propool.tile([128, 256], F32, tag="erows")
                    nc.gpsimd.indirect_dma_start(
                        out=erows[:],
                        out_offset=None,
                        in_=embed_d[:],
                        in_offset=bass.IndirectOffsetOnAxis(
                            ap=idx_t[:, th:th + 1], axis=0),
                    )
                    for kc in range(2):
                        ptr = propspool.tile([128, 128], F32, tag="ptr")
                        nc.tensor.transpose(
                            out=ptr[:], in_=erows[:, kc * 128:(kc + 1) * 128],
                            identity=eye_t[:],
                        )
                        nc.vector.tensor_copy(out=embT_t[:, kc, th, :], in_=ptr[:])

                for th in range(2):
                    psx = propspool.tile([128, 1024], F32, tag="psx")
                    for n2 in range(2):
                        sl = slice(n2 * 512, (n2 + 1) * 512)
                        for kc in range(2):
                            nc.tensor.matmul(
                                psx[:, sl], embT_t[:, kc, th, :], wx_t[:, kc, sl],
                                start=(kc == 0), stop=False,
                            )
                        nc.tensor.matmul(
                            psx[:, sl], ones_t[:], bx_t[:, sl],
                            start=False, stop=True,
                        )
                    nc.vector.tensor_copy(out=xp_t[:, th, :], in_=psx[:])

                for b in range(NB):
                    hb = propool.tile([128, 8, 128], F32R, tag="hb")
                    for kc in range(8):
                        nc.sync.dma_start(out=hb[:, kc, :], in_=hidT_d[b, kc])
                    for hc in range(4):
                        pse = propspool.tile([128, 128], F32, tag="pse")
                        for kc in range(8):
                            nc.tensor.matmul(
                                pse[:], wenc_t[:, kc, hc * 128:(hc + 1) * 128],
                                hb[:, kc, :],
                                start=(kc == 0), stop=(kc == 7),
                            )
                        nc.vector.tensor_scalar_add(
                            encT_t[:, b * 512 + hc * 128: b * 512 + (hc + 1) * 128],
                            pse[:], bebp_t[:, hc:hc + 1],
                        )
                    psp = propspool.tile([128, 1024], F32, tag="psp")
                    for n2 in range(2):
                        sl = slice(n2 * 512, (n2 + 1) * 512)
                        for kc in range(8):
                            nc.tensor.matmul(
                                psp[:, sl], hb[:, kc, :], wc_t[:, kc, sl],
                                start=(kc == 0), stop=(kc == 7),
                            )
                    nc.vector.tensor_copy(out=pc_t[:, b, :], in_=psp[:])

            # ---- state ----
            sT = spool.tile([128, 16], F32R, tag="sT")
            nc.sync.dma_start(out=sT[:], in_=s0_d[:])
            atb = cpool.tile([128, 16], F32R)  # block-diag alphaT: col 5b = alpha_b
            nc.sync.dma_start(out=atb[:], in_=z16_d[:])
            if debug:
                nc.sync.dma_start(out=dbg_enc[:], in_=encT_t[:])
                nc.gpsimd.dma_start(out=dbg_pc[:], in_=pc_t[:])
                nc.gpsimd.dma_start(out=dbg_xp[:], in_=xp_t[:])
                nc.gpsimd.dma_start(out=dbg_emb[:], in_=embT_t[:])

            # ---- recurrence ----
            with (
                tc.tile_pool(name="psA", bufs=3, space="PSUM") as psA,
                tc.tile_pool(name="psB", bufs=2, space="PSUM") as psB,
            ):
                stg = None
                for t in range(T):
                    tm, th = t % 32, t // 32
                    # 1. q matmuls
                    psc = psA.tile([128, 1024], F32, tag="psc")
                    psq = psB.tile([128, 16], F32, tag="misc")
                    for hc in range(4):
                        for kc in range(4):
                            nc.tensor.matmul(
                                psq[:, 4 * hc:4 * hc + 4],
                                ws_t[:, kc, hc * 128:(hc + 1) * 128],
                                sT[:, 4 * kc:4 * kc + 4],
                                start=(kc == 0), stop=(kc == 3),
                                skip_group_check=True,
                            )
                    qT = wpool.tile([128, 16], F32, tag="qT")
                    nc.vector.tensor_copy(out=qT[:], in_=psq[:])
                    # 2. hT = tanh(encT + qT)
                    hT = wpool.tile([128, 2048], BF16, tag="hT")
                    hpre = wpool.tile([128, 2048], F32, tag="hpre")
                    for b in range(NB):
                        for hc in range(4):
                            sl = slice(b * 512 + hc * 128, b * 512 + (hc + 1) * 128)
                            nc.vector.tensor_scalar_add(
                                hpre[:, sl], encT_t[:, sl],
                                qT[:, 4 * hc + b:4 * hc + b + 1],
                            )
                        nc.scalar.activation(
                            hT[:, b * 512:(b + 1) * 512],
                            hpre[:, b * 512:(b + 1) * 512], AF.Tanh,
                        )
                    # 3. score + alpha
                    pssc = psB.tile([1, 512], F32, tag="misc")
                    for b in range(NB):
                        for hc in range(4):
                            nc.tensor.matmul(
                                pssc[0:1, b * 128:(b + 1) * 128],
                                watt_t[:, hc:hc + 1],
                                hT[:, b * 512 + hc * 128: b * 512 + (hc + 1) * 128],
                                start=(hc == 0), stop=(hc == 3),
                                skip_group_check=True,
                            )
                    arow = wpool.tile([1, 512], F32, tag="arow")
                    sums = wpool.tile([1, 4], F32, tag="sums")
                    for b in range(NB):
                        nc.scalar.activation(
                            arow[0:1, b * 128:(b + 1) * 128],
                            pssc[0:1, b * 128:(b + 1) * 128],
                            AF.Exp, accum_out=sums[0:1, b:b + 1],
                        )
                    if debug and t == 0:
                        scrow = wpool.tile([1, 512], F32, tag="scrow")
                        nc.vector.tensor_copy(out=scrow[:], in_=pssc[:])
                        nc.sync.dma_start(out=dbg_sc[:], in_=scrow[:])
                        nc.sync.dma_start(out=dbg_ar[:], in_=arow[:])
                    rsum = wpool.tile([1, 4], F32R, tag="rsum")
                    nc.vector.reciprocal(out=rsum[:], in_=sums[:])
                    psrs = psB.tile([128, 4], F32, tag="misc")
                    nc.tensor.matmul(psrs[:], ones_t[:], rsum[:])
                    rsb = wpool.tile([128, 4], F32, tag="rsb")
                    nc.vector.tensor_copy(out=rsb[:], in_=psrs[:])
                    psa = psB.tile([128, 4], F32, tag="misc")
                    for b in range(NB):
                        nc.tensor.transpose(
                            out=psa[:, b:b + 1],
                            in_=arow[0:1, b * 128:(b + 1) * 128],
                            identity=eye_t[0:1, 0:1])
                    for b in range(NB):
                        nc.vector.tensor_mul(
                            atb[:, 5 * b:5 * b + 1], psa[:, b:b + 1],
                            rsb[:, b:b + 1])
                    if debug and t == 0:
                        nc.gpsimd.dma_start(out=dbg_atb[:], in_=atb[:])
                    # 4. r/st matmuls
                    for g in (1, 2):
                        o = psc[0:4, (g - 1) * 512:g * 512]
                        xsl = slice((g - 1) * 512, g * 512)
                        for kc in range(4):
                            nc.tensor.matmul(
                                o, sT[:, 4 * kc:4 * kc + 4],
                                ws_t[:, kc, g * 512:(g + 1) * 512],
                                start=(kc == 0), stop=False,
                                skip_group_check=True,
                            )
                        nc.tensor.matmul(
                            o, eyer_t[:, 4 * tm:4 * tm + 4], xp_t[:, th, xsl],
                            start=False, stop=False,
                            skip_group_check=True,
                        )
                    for g in (1, 2):
                        o = psc[0:4, (g - 1) * 512:g * 512]
                        xsl = slice((g - 1) * 512, g * 512)
                        for bp in range(NB):
                            nc.tensor.matmul(
                                o, atb[:, 4 * bp:4 * bp + 4], pc_t[:, bp, xsl],
                                start=False, stop=(bp == NB - 1),
                                skip_group_check=True,
                            )
                    # 5. gates
                    trows = wpool.tile([4, 512], F32, tag="trows")
                    strows = wpool.tile([4, 512], F32, tag="strows")
                    nc.scalar.activation(trows[:], psc[0:4, 0:512], AF.Tanh, scale=0.5)
                    nc.scalar.activation(strows[:], psc[0:4, 512:1024], AF.Tanh)
                    psg = psB.tile([128, 32], F32, tag="misc")
                    for hc in range(4):
                        nc.tensor.transpose(
                            out=psg[:, 4 * hc:4 * hc + 4],
                            in_=trows[:, hc * 128:(hc + 1) * 128],
                            identity=eye_t[0:4, 0:4],
                        )
                        nc.tensor.transpose(
                            out=psg[:, 16 + 4 * hc:16 + 4 * hc + 4],
                            in_=strows[:, hc * 128:(hc + 1) * 128],
                            identity=eye_t[0:4, 0:4],
                        )
                    # 6. update: r = 0.5*tau + 0.5 ; s' = s + r*(st - s)
                    r_t = wpool.tile([128, 16], F32, tag="r_t")
                    nc.vector.tensor_scalar(
                        r_t[:], psg[:, 0:16], 0.5, 0.5,
                        mybir.AluOpType.mult, mybir.AluOpType.add)
                    d_t = wpool.tile([128, 16], F32, tag="d_t")
                    nc.vector.tensor_sub(d_t[:], psg[:, 16:32], sT[:])
                    p_t = wpool.tile([128, 16], F32, tag="p_t")
                    nc.vector.tensor_mul(p_t[:], r_t[:], d_t[:])
                    sTn = spool.tile([128, 16], F32R, tag="sT")
                    nc.vector.tensor_add(sTn[:], sT[:], p_t[:])
                    sT = sTn
                    # stage output (+0.5*sentiment), DMA every 8 steps
                    if t % 8 == 0:
                        stg = stpool.tile([128, 8, 16], F32, tag="stg")
                    nc.vector.tensor_add(stg[:, t % 8, :], sT[:], sh_t[:])
                    if t % 8 == 7:
                        nc.sync.dma_start(
                            out=dec_d[t - 7:t + 1].rearrange("t p j -> p t j"),
                            in_=stg[:])
    return nc


def host_prep(content, sentiment, hiddens, target, embed,
              W_enc, b_enc, W_prev, b_prev, W_att,
              Wi_g, bi_g, Wh_g, bh_g, Wc_g, bc_g,
              Wi, bi, Wh, bh, Wc, bc, core):
    """Build the per-core input map (batches 4*core .. 4*core+3)."""
    import numpy as np
    import ml_dtypes
    bs = slice(4 * core, 4 * core + 4)
    hid = hiddens[bs]                                    # [4,128,1024]
    hidT = np.ascontiguousarray(hid.transpose(0, 2, 1).reshape(4, 8, 128, 128))
    ws = np.concatenate([W_prev, Wh_g[:, :512], Wh], axis=1)      # [512,1536]
    ws = np.ascontiguousarray(ws.reshape(4, 128, 1536))
    wcm = np.concatenate([Wc_g[:, :512], Wc], axis=1)             # [1024,1024]
    wcm = np.ascontiguousarray(wcm.reshape(8, 128, 1024))
    wenc = np.ascontiguousarray(W_enc.reshape(8, 128, 512))
    wx = np.concatenate([Wi_g[:, :512], Wi], axis=1)              # [256,1024]
    wx = np.ascontiguousarray(wx.reshape(2, 128, 1024))
    bx = (np.concatenate([bi_g[:512] + bh_g[:512] + bc_g[:512], bi + bh + bc])
          .reshape(1, 1024))
    bebp = np.ascontiguousarray((b_enc + b_prev).reshape(4, 128).T)
    watt = np.ascontiguousarray(
        W_att[:, 0].reshape(4, 128).T).astype(ml_dtypes.bfloat16)
    s0 = np.zeros((128, 16), np.float32)
    sh = np.zeros((128, 16), np.float32)
    for b in range(4):
        for kc in range(4):
            s0[:, 4 * kc + b] = content[4 * core + b, kc * 128:(kc + 1) * 128]
            sh[:, 4 * kc + b] = 0.5 * sentiment[4 * core + b, kc * 128:(kc + 1) * 128]
    idx = np.zeros((128, 2), np.int32)
    for th in range(2):
        for tm in range(32):
            for b in range(4):
                idx[tm * 4 + b, th] = target[4 * core + b, th * 32 + tm]
    eye = np.eye(128, dtype=np.float32)
    return dict(
        hidT=hidT.astype(np.float32), ws=ws.astype(np.float32),
        wc=wcm.astype(np.float32), wenc=wenc.astype(np.float32),
        wx=wx.astype(np.float32), bx=bx.astype(np.float32),
        bebp=bebp.astype(np.float32), watt=watt,
        s0=s0, sh=sh, idx=idx, eye=eye, eyer=eye.copy(),
        onesr=np.ones((1, 128), np.float32), z16=np.zeros((128, 16), np.float32),
        embed=embed.astype(np.float32),
    )


def dec_from_out(decT):
    """decT [64,128,16] -> A_local [4, 64, 512] (dec_h + 0.5*sent)."""
    import numpy as np
    out = np.zeros((4, 64, 512), np.float32)
    for b in range(4):
        for kc in range(4):
            out[b, :, kc * 128:(kc + 1) * 128] = decT[:, :, 4 * kc + b]
    return out


# ---------------- projection (launch B) ----------------



KC = 4          # contraction chunks (H=512)
MV = 32         # vocab 128-blocks per shard (4096 padded)
NT = 4          # BT=2048 -> 4 chunks of 512
BT = 2048
F32R = mybir.dt.float32r
F32 = mybir.dt.float32
FP8 = mybir.dt.float8e4


def build_proj():
    """3-term fp8e4m3 DoubleRow split: A8@W8 + (A8/16)@(16*Wlo) + Alo@W8.
    Each term contracts K=256/instr at 0.5 cyc/row; bf16 output."""
    nc = bass.Bass()
    at_d = nc.dram_tensor("at8", [3, 2, 128, 2, BT], FP8, kind="ExternalInput")
    wo_d = nc.dram_tensor("wo8", [2, 2, 128, 2, MV * 128], FP8,
                          kind="ExternalInput")
    bo_d = nc.dram_tensor("bo", [128, MV], F32, kind="ExternalInput")
    out_d = nc.dram_tensor("outT", [MV * 128, BT], BF16, kind="ExternalOutput")
    outv = out_d.rearrange("(m p) n -> m p n", p=128)
    TERMS = [(0, 0), (1, 1), (2, 0)]   # (activation variant, weight variant)

    with tile.TileContext(nc) as tc:
        with (
            nc.allow_low_precision(reason="fp8 split validated vs reference"),
            tc.tile_pool(name="weights", bufs=1) as wpool,
            tc.tile_pool(name="outs", bufs=6) as opool,
            tc.tile_pool(name="psum", bufs=8, space="PSUM") as ppool,
        ):
            at_t = wpool.tile([128, 3, 2, 2, BT], FP8)
            wo_t = wpool.tile([128, 2, 2, 2, MV * 128], FP8)
            bo_t = wpool.tile([128, MV], F32)
            nc.sync.dma_start(out=bo_t[:], in_=bo_d[:])
            for v in range(3):
                for kc2 in range(2):
                    for i in range(2):
                        nc.sync.dma_start(out=at_t[:, v, kc2, i, :],
                                          in_=at_d[v, kc2, :, i, :])
            for w in range(2):
                for kc2 in range(2):
                    for i in range(2):
                        for mg in range(2):
                            sl = slice(mg * 2048, (mg + 1) * 2048)
                            nc.sync.dma_start(out=wo_t[:, w, kc2, i, sl],
                                              in_=wo_d[w, kc2, :, i, sl])

            for m in range(MV):
                for nt in range(NT):
                    ps = ppool.tile([128, 512], F32)
                    n_mm = 0
                    for v, w in TERMS:
                        for kc2 in range(2):
                            n_mm += 1
                            nc.tensor.matmul(
                                ps[:],
                                wo_t[:, w, kc2, :, m * 128:(m + 1) * 128],
                                at_t[:, v, kc2, :, nt * 512:(nt + 1) * 512],
                                start=(n_mm == 1),
                                stop=(n_mm == 6),
                                perf_mode=mybir.MatmulPerfMode.DoubleRow,
                            )
                    ot = opool.tile([128, 512], BF16)
                    nc.vector.tensor_scalar_add(ot[:], ps[:], bo_t[:, m:m + 1])
                    nc.sync.dma_start(
                        out=outv[m, :, nt * 512:(nt + 1) * 512], in_=ot[:]
                    )
    return nc


# ---------------- orchestration ----------------

_B, _T, _H, _V = 32, 64, 512, 32000
_VS = _V // 8          # vocab shard
_VP = 4096             # padded shard

_cache = {}
LAST_PERF = {}


def _trace_flag():
    import os
    return bool(int(os.environ.get("BASS_KERNEL_TRACE", "0")))


def _get_progs():
    if "rec" not in _cache:
        _apply_walrus_patch()
        _cache["rec"] = build_rec()
        _cache["proj"] = build_proj()
    return _cache["rec"], _cache["proj"]


def kernel(content, sentiment, hiddens, target, embed,
           W_enc, b_enc, W_prev, b_prev, W_att, b_att,
           Wi_g, bi_g, Wh_g, bh_g, Wc_g, bc_g,
           Wi, bi, Wh, bh, Wc, bc, W_out, b_out):
    from concourse.bass_utils import run_bass_kernel_spmd

    content = np.asarray(content, np.float32)
    sentiment = np.asarray(sentiment, np.float32)
    hiddens = np.asarray(hiddens, np.float32)
    target = np.asarray(target, np.int32)
    embed = np.asarray(embed, np.float32)

    rec_nc, proj_nc = _get_progs()
    trace = _trace_flag()

    in_maps_a = [
        host_prep(content, sentiment, hiddens, target, embed,
                  np.asarray(W_enc, np.float32), np.asarray(b_enc, np.float32),
                  np.asarray(W_prev, np.float32), np.asarray(b_prev, np.float32),
                  np.asarray(W_att, np.float32),
                  np.asarray(Wi_g, np.float32), np.asarray(bi_g, np.float32),
                  np.asarray(Wh_g, np.float32), np.asarray(bh_g, np.float32),
                  np.asarray(Wc_g, np.float32), np.asarray(bc_g, np.float32),
                  np.asarray(Wi, np.float32), np.asarray(bi, np.float32),
                  np.asarray(Wh, np.float32), np.asarray(bh, np.float32),
                  np.asarray(Wc, np.float32), np.asarray(bc, np.float32), core)
        for core in range(8)
    ]
    res_a = run_bass_kernel_spmd(rec_nc, in_maps_a, core_ids=list(range(8)),
                                 trace=trace)
    A = np.empty((_B, _T, _H), np.float32)
    for core in range(8):
        A[4 * core:4 * core + 4] = dec_from_out(res_a.results[core]["decT"])

    F8 = ml_dtypes.float8_e4m3fn

    def _drpack(M):   # [512, N] f32 -> DoubleRow fp8 pack [2, 128, 2, N]
        return np.ascontiguousarray(
            M.reshape(2, 2, 128, -1).transpose(0, 2, 1, 3)).astype(F8)

    At = A.reshape(_B * _T, _H).T.astype(np.float32)    # [512, 2048]
    A8f = At.astype(F8).astype(np.float32)
    at8 = np.stack([_drpack(At), _drpack(At / 16.0), _drpack(At - A8f)])
    W_out = np.asarray(W_out, np.float32)
    b_out = np.asarray(b_out, np.float32)
    in_maps_b = []
    for core in range(8):
        wsh = np.zeros((_H, _VP), np.float32)
        wsh[:, :_VS] = W_out[:, core * _VS:(core + 1) * _VS]
        bsh = np.zeros(_VP, np.float32)
        bsh[:_VS] = 1.5 * b_out[core * _VS:(core + 1) * _VS]
        W8f = wsh.astype(F8).astype(np.float32)
        wo8 = np.stack([_drpack(wsh), _drpack(16.0 * (wsh - W8f))])
        in_maps_b.append(dict(
            at8=at8,
            wo8=wo8,
            bo=np.ascontiguousarray(bsh.reshape(_VP // 128, 128).T),
        ))
    res_b = run_bass_kernel_spmd(proj_nc, in_maps_b, core_ids=list(range(8)),
                                 trace=trace)
    out = np.empty((_B, _T, _V), np.float32)
    for core in range(8):
        sh = res_b.results[core]["outT"][:_VS]          # [4000, 2048] bf16
        out[:, :, core * _VS:(core + 1) * _VS] = (
            sh.astype(np.float32).T.reshape(_B, _T, _VS))

    if trace:
        LAST_PERF["rec_ns"] = res_a.exec_time_ns
        LAST_PERF["proj_ns"] = res_b.exec_time_ns
    return out



# revision 13
# speedup vs baseline: 1.0471x; 1.0047x over previous
"""Trainium2 Bass kernel for nn_Decoder_80315888436037.

Two SPMD launches on 8 NeuronCores:
  A) attention+GRU recurrence, data-parallel over batch (4 batches/core),
     f32r matmuls (tf32-like PE mode).
  B) vocab projection (dec_h+0.5*sent) @ W_out + 1.5*b_out, vocab-sharded
     (4000 cols/core). Error-compensated fp8e4m3 DoubleRow matmuls
     (A8@W8 + (A8/16)@(16*Wlo) + Alo@W8, K=256/instr at 0.5 cyc/row),
     bf16 logits output upcast on host. Verified rel err 3.1e-03.
Host work between launches is only gather/reshape/transpose of activations.
"""

import numpy as np
import ml_dtypes

import concourse.bass as bass
import concourse.mybir as mybir
import concourse.tile as tile



import json

import concourse.bass_utils as _bu
import concourse.bass2jax as _b2j

_MAX_W = 1
_MAX_U = 1
_orig_compile_bir_kernel = _bu.compile_bir_kernel


def _split_sync(bir_json: bytes) -> bytes:
    m = json.loads(bir_json)
    uid = [0]

    def carrier(engine, debug, waits=None, updates=None):
        uid[0] += 1
        return {
            "debug": debug,
            "engine": engine,
            "ins": [],
            "name": f"WSPLIT-{uid[0]}",
            "opcode": "EventSemaphore",
            "outs": [],
            "sync_info": {"on_update": updates or [], "on_wait": waits or []},
        }

    changed = False
    for fn in m.get("functions", []):
        for bb in fn.get("blocks", []):
            out = []
            for inst in bb.get("instructions", []):
                si = inst.get("sync_info")
                if not si:
                    out.append(inst)
                    continue
                waits = si.get("on_wait") or []
                pre = []
                if len(waits) > _MAX_W:
                    changed = True
                    keep = waits[-_MAX_W:]
                    for w in waits[:-_MAX_W]:
                        pre.append(carrier(inst["engine"], inst.get("debug", 0), waits=[w]))
                    si["on_wait"] = keep
                out.extend(pre)
                out.append(inst)
            bb["instructions"] = out
    if not changed:
        return bir_json
    return json.dumps(m).encode()


def _patched_compile_bir_kernel(bir_json, tmpdir, neff_name="file.neff"):
    if isinstance(bir_json, str):
        bir_json = bir_json.encode()
    return _orig_compile_bir_kernel(_split_sync(bir_json), tmpdir, neff_name=neff_name)


def _apply_walrus_patch():
    _bu.compile_bir_kernel = _patched_compile_bir_kernel
    _b2j.compile_bir_kernel = _patched_compile_bir_kernel


# ---------------- recurrence (launch A) ----------------



F32R = mybir.dt.float32r
F32 = mybir.dt.float32
BF16 = mybir.dt.bfloat16
I32 = mybir.dt.int32
AF = mybir.ActivationFunctionType

T = 64
NB = 4  # batches per core


def build_rec(debug=False):
    nc = bass.Bass()
    hidT_d = nc.dram_tensor("hidT", [NB, 8, 128, 128], F32R, kind="ExternalInput")
    ws_d = nc.dram_tensor("ws", [4, 128, 1536], F32R, kind="ExternalInput")
    wc_d = nc.dram_tensor("wc", [8, 128, 1024], F32R, kind="ExternalInput")
    wenc_d = nc.dram_tensor("wenc", [8, 128, 512], F32R, kind="ExternalInput")
    wx_d = nc.dram_tensor("wx", [2, 128, 1024], F32R, kind="ExternalInput")
    bx_d = nc.dram_tensor("bx", [1, 1024], F32R, kind="ExternalInput")
    bebp_d = nc.dram_tensor("bebp", [128, 4], F32, kind="ExternalInput")
    watt_d = nc.dram_tensor("watt", [128, 4], BF16, kind="ExternalInput")
    s0_d = nc.dram_tensor("s0", [128, 16], F32R, kind="ExternalInput")
    sh_d = nc.dram_tensor("sh", [128, 16], F32, kind="ExternalInput")
    idx_d = nc.dram_tensor("idx", [128, 2], I32, kind="ExternalInput")
    eye_d = nc.dram_tensor("eye", [128, 128], F32, kind="ExternalInput")
    eyer_d = nc.dram_tensor("eyer", [128, 128], F32R, kind="ExternalInput")
    ones_d = nc.dram_tensor("onesr", [1, 128], F32R, kind="ExternalInput")
    z16_d = nc.dram_tensor("z16", [128, 16], F32R, kind="ExternalInput")
    embed_d = nc.dram_tensor("embed", [32000, 256], F32, kind="ExternalInput")
    dec_d = nc.dram_tensor("decT", [T, 128, 16], F32, kind="ExternalOutput")
    if debug:
        dbg_enc = nc.dram_tensor("dbg_enc", [128, 2048], F32, kind="ExternalOutput")
        dbg_pc = nc.dram_tensor("dbg_pc", [128, 4, 1024], F32, kind="ExternalOutput")
        dbg_xp = nc.dram_tensor("dbg_xp", [128, 2, 1024], F32, kind="ExternalOutput")
        dbg_emb = nc.dram_tensor("dbg_emb", [128, 2, 2, 128], F32, kind="ExternalOutput")
        dbg_q = nc.dram_tensor("dbg_q", [128, 16], F32, kind="ExternalOutput")
        dbg_sc = nc.dram_tensor("dbg_sc", [1, 512], F32, kind="ExternalOutput")
        dbg_ar = nc.dram_tensor("dbg_ar", [1, 512], F32, kind="ExternalOutput")
        dbg_atb = nc.dram_tensor("dbg_atb", [128, 16], F32, kind="ExternalOutput")
        dbg_gates = nc.dram_tensor("dbg_gates", [4, 1536], F32, kind="ExternalOutput")

    with tile.TileContext(nc) as tc:
        with (
            nc.allow_low_precision(reason="float32r tiles carry full fp32 bits"),
            tc.tile_pool(name="const", bufs=1) as cpool,
            tc.tile_pool(name="state", bufs=2) as spool,
            tc.tile_pool(name="work", bufs=2) as wpool,
            tc.tile_pool(name="stage", bufs=2) as stpool,
        ):
            # ---- resident constants/weights ----
            ws_t = cpool.tile([128, 4, 1536], F32R)
            wc_t = cpool.tile([128, 8, 1024], F32R)
            wenc_t = cpool.tile([128, 8, 512], F32R)
            wx_t = cpool.tile([128, 2, 1024], F32R)
            bx_t = cpool.tile([1, 1024], F32R)
            bebp_t = cpool.tile([128, 4], F32)
            watt_t = cpool.tile([128, 4], BF16)
            sh_t = cpool.tile([128, 16], F32)
            idx_t = cpool.tile([128, 2], I32)
            eye_t = cpool.tile([128, 128], F32)
            eyer_t = cpool.tile([128, 128], F32R)
            ones_t = cpool.tile([1, 128], F32R)
            # prologue-critical loads first (gather idx, X-pack, enc/PC
            # weights); step-only weights (ws, watt, ...) issued last so the
            # DMA queue doesn't delay the first prologue matmuls
            nc.sync.dma_start(out=idx_t[:], in_=idx_d[:])
            nc.sync.dma_start(out=eye_t[:], in_=eye_d[:])
            for kc in range(2):
                nc.sync.dma_start(out=wx_t[:, kc, :], in_=wx_d[kc])
            nc.sync.dma_start(out=bx_t[:], in_=bx_d[:])
            nc.sync.dma_start(out=ones_t[:], in_=ones_d[:])
            for kc in range(8):
                nc.sync.dma_start(out=wenc_t[:, kc, :], in_=wenc_d[kc])
                nc.sync.dma_start(out=wc_t[:, kc, :], in_=wc_d[kc])
            nc.sync.dma_start(out=bebp_t[:], in_=bebp_d[:])
            for kc in range(4):
                nc.sync.dma_start(out=ws_t[:, kc, :], in_=ws_d[kc])
            nc.sync.dma_start(out=watt_t[:], in_=watt_d[:])
            nc.sync.dma_start(out=sh_t[:], in_=sh_d[:])
            nc.sync.dma_start(out=eyer_t[:], in_=eyer_d[:])

            encT_t = cpool.tile([128, 2048], F32)     # (b, hc, s)
            pc_t = cpool.tile([128, 4, 1024], F32R)   # [s, b, n]
            embT_t = cpool.tile([128, 2, 2, 128], F32R)
            xp_t = cpool.tile([128, 2, 1024], F32R)

            # ---- prologue: gather, X_pack, encT, P_c ----
            with (
                tc.tile_pool(name="pro", bufs=2) as propool,
                tc.tile_pool(name="props", bufs=1, space="PSUM") as propspool,
            ):
                for th in range(2):
                    erows = propool.tile([128, 256], F32, tag="erows")
                    nc.gpsimd.indirect_dma_start(
                        out=erows[:],
                        out_offset=None,
                        in_=embed_d[:],
                        in_offset=bass.IndirectOffsetOnAxis(
                            ap=idx_t[:, th:th + 1], axis=0),
                    )
                    for kc in range(2):
                        ptr = propspool.tile([128, 128], F32, tag="ptr")
                        nc.tensor.transpose(
                            out=ptr[:], in_=erows[:, kc * 128:(kc + 1) * 128],
                            identity=eye_t[:],
                        )
                        nc.vector.tensor_copy(out=embT_t[:, kc, th, :], in_=ptr[:])

                for th in range(2):
                    psx = propspool.tile([128, 1024], F32, tag="psx")
                    for n2 in range(2):
                        sl = slice(n2 * 512, (n2 + 1) * 512)
                        for kc in range(2):
                            nc.tensor.matmul(
                                psx[:, sl], embT_t[:, kc, th, :], wx_t[:, kc, sl],
                                start=(kc == 0), stop=False,
                            )
                        nc.tensor.matmul(
                            psx[:, sl], ones_t[:], bx_t[:, sl],
                            start=False, stop=True,
                        )
                    nc.vector.tensor_copy(out=xp_t[:, th, :], in_=psx[:])

                for b in range(NB):
                    hb = propool.tile([128, 8, 128], F32R, tag="hb")
                    for kc in range(8):
                        nc.sync.dma_start(out=hb[:, kc, :], in_=hidT_d[b, kc])
                    for hc in range(4):
                        pse = propspool.tile([128, 128], F32, tag="pse")
                        for kc in range(8):
                            nc.tensor.matmul(
                                pse[:], wenc_t[:, kc, hc * 128:(hc + 1) * 128],
                                hb[:, kc, :],
                                start=(kc == 0), stop=(kc == 7),
                            )
                        nc.vector.tensor_scalar_add(
                            encT_t[:, b * 512 + hc * 128: b * 512 + (hc + 1) * 128],
                            pse[:], bebp_t[:, hc:hc + 1],
                        )
                    psp = propspool.tile([128, 1024], F32, tag="psp")
                    for n2 in range(2):
                        sl = slice(n2 * 512, (n2 + 1) * 512)
                        for kc in range(8):
                            nc.tensor.matmul(
                                psp[:, sl], hb[:, kc, :], wc_t[:, kc, sl],
                                start=(kc == 0), stop=(kc == 7),
                            )
                    nc.vector.tensor_copy(out=pc_t[:, b, :], in_=psp[:])

            # ---- state ----
            sT = spool.tile([128, 16], F32R, tag="sT")
            nc.sync.dma_start(out=sT[:], in_=s0_d[:])
            atb = cpool.tile([128, 16], F32R)  # block-diag alphaT: col 5b = alpha_b
            nc.sync.dma_start(out=atb[:], in_=z16_d[:])
            if debug:
                nc.sync.dma_start(out=dbg_enc[:], in_=encT_t[:])
                nc.gpsimd.dma_start(out=dbg_pc[:], in_=pc_t[:])
                nc.gpsimd.dma_start(out=dbg_xp[:], in_=xp_t[:])
                nc.gpsimd.dma_start(out=dbg_emb[:], in_=embT_t[:])

            # ---- recurrence ----
            with (
                tc.tile_pool(name="psA", bufs=3, space="PSUM") as psA,
                tc.tile_pool(name="psB", bufs=2, space="PSUM") as psB,
            ):
                stg = None
                for t in range(T):
                    tm, th = t % 32, t // 32
                    # 1. q matmuls
                    psc = psA.tile([128, 1024], F32, tag="psc")
                    psq = psB.tile([128, 16], F32, tag="misc")
                    for hc in range(4):
                        for kc in range(4):
                            nc.tensor.matmul(
                                psq[:, 4 * hc:4 * hc + 4],
                                ws_t[:, kc, hc * 128:(hc + 1) * 128],
                                sT[:, 4 * kc:4 * kc + 4],
                                start=(kc == 0), stop=(kc == 3),
                                skip_group_check=True,
                            )
                    qT = wpool.tile([128, 16], F32, tag="qT")
                    nc.vector.tensor_copy(out=qT[:], in_=psq[:])
                    # 2. hT = tanh(encT + qT)
                    hT = wpool.tile([128, 2048], BF16, tag="hT")
                    hpre = wpool.tile([128, 2048], F32, tag="hpre")
                    for b in range(NB):
                        for hc in range(4):
                            sl = slice(b * 512 + hc * 128, b * 512 + (hc + 1) * 128)
                            nc.vector.tensor_scalar_add(
                                hpre[:, sl], encT_t[:, sl],
                                qT[:, 4 * hc + b:4 * hc + b + 1],
                            )
                        nc.scalar.activation(
                            hT[:, b * 512:(b + 1) * 512],
                            hpre[:, b * 512:(b + 1) * 512], AF.Tanh,
                        )
                    # 3. score + alpha
                    pssc = psB.tile([1, 512], F32, tag="misc")
                    for b in range(NB):
                        for hc in range(4):
                            nc.tensor.matmul(
                                pssc[0:1, b * 128:(b + 1) * 128],
                                watt_t[:, hc:hc + 1],
                                hT[:, b * 512 + hc * 128: b * 512 + (hc + 1) * 128],
                                start=(hc == 0), stop=(hc == 3),
                                skip_group_check=True,
                            )
                    arow = wpool.tile([1, 512], F32, tag="arow")
                    sums = wpool.tile([1, 4], F32, tag="sums")
                    for b in range(NB):
                        nc.scalar.activation(
                            arow[0:1, b * 128:(b + 1) * 128],
                            pssc[0:1, b * 128:(b + 1) * 128],
                            AF.Exp, accum_out=sums[0:1, b:b + 1],
                        )
                    if debug and t == 0:
                        scrow = wpool.tile([1, 512], F32, tag="scrow")
                        nc.vector.tensor_copy(out=scrow[:], in_=pssc[:])
                        nc.sync.dma_start(out=dbg_sc[:], in_=scrow[:])
                        nc.sync.dma_start(out=dbg_ar[:], in_=arow[:])
                    rsum = wpool.tile([1, 4], F32R, tag="rsum")
                    nc.vector.reciprocal(out=rsum[:], in_=sums[:])
                    psrs = psB.tile([128, 4], F32, tag="misc")
                    nc.tensor.matmul(psrs[:], ones_t[:], rsum[:])
                    rsb = wpool.tile([128, 4], F32, tag="rsb")
                    nc.vector.tensor_copy(out=rsb[:], in_=psrs[:])
                    psa = psB.tile([128, 4], F32, tag="misc")
                    for b in range(NB):
                        nc.tensor.transpose(
                            out=psa[:, b:b + 1],
                            in_=arow[0:1, b * 128:(b + 1) * 128],
                            identity=eye_t[0:1, 0:1])
                    for b in range(NB):
                        nc.vector.tensor_mul(
                            atb[:, 5 * b:5 * b + 1], psa[:, b:b + 1],
                            rsb[:, b:b + 1])
                    if debug and t == 0:
                        nc.gpsimd.dma_start(out=dbg_atb[:], in_=atb[:])
                    # 4. r/st matmuls
                    for g in (1, 2):
                        o = psc[0:4, (g - 1) * 512:g * 512]
                        xsl = slice((g - 1) * 512, g * 512)
                        for kc in range(4):
                            nc.tensor.matmul(
                                o, sT[:, 4 * kc:4 * kc + 4],
                                ws_t[:, kc, g * 512:(g + 1) * 512],
                                start=(kc == 0), stop=False,
                                skip_group_check=True,
                            )
                        nc.tensor.matmul(
                            o, eyer_t[:, 4 * tm:4 * tm + 4], xp_t[:, th, xsl],
                            start=False, stop=False,
                            skip_group_check=True,
                        )
                    for g in (1, 2):
                        o = psc[0:4, (g - 1) * 512:g * 512]
                        xsl = slice((g - 1) * 512, g * 512)
                        for bp in range(NB):
                            nc.tensor.matmul(
                                o, atb[:, 4 * bp:4 * bp + 4], pc_t[:, bp, xsl],
                                start=False, stop=(bp == NB - 1),
                                skip_group_check=True,
                            )
                    # 5. gates
                    trows = wpool.tile([4, 512], F32, tag="trows")
                    strows = wpool.tile([4, 512], F32, tag="strows")
                    nc.scalar.activation(trows[:], psc[0:4, 0:512], AF.Tanh, scale=0.5)
                    nc.scalar.activation(strows[:], psc[0:4, 512:1024], AF.Tanh)
                    psg = psB.tile([128, 32], F32, tag="misc")
                    for hc in range(4):
                        nc.tensor.transpose(
                            out=psg[:, 4 * hc:4 * hc + 4],
                            in_=trows[:, hc * 128:(hc + 1) * 128],
                            identity=eye_t[0:4, 0:4],
                        )
                        nc.tensor.transpose(
                            out=psg[:, 16 + 4 * hc:16 + 4 * hc + 4],
                            in_=strows[:, hc * 128:(hc + 1) * 128],
                            identity=eye_t[0:4, 0:4],
                        )
                    # 6. update: r = 0.5*tau + 0.5 ; s' = s + r*(st - s)
                    r_t = wpool.tile([128, 16], F32, tag="r_t")
                    nc.vector.tensor_scalar(
                        r_t[:], psg[:, 0:16], 0.5, 0.5,
                        mybir.AluOpType.mult, mybir.AluOpType.add)
                    d_t = wpool.tile([128, 16], F32, tag="d_t")
                    nc.vector.tensor_sub(d_t[:], psg[:, 16:32], sT[:])
                    p_t = wpool.tile([128, 16], F32, tag="p_t")
                    nc.vector.tensor_mul(p_t[:], r_t[:], d_t[:])
                    sTn = spool.tile([128, 16], F32R, tag="sT")
                    nc.vector.tensor_add(sTn[:], sT[:], p_t[:])
                    sT = sTn
                    # stage output (+0.5*sentiment), DMA every 8 steps
                    if t % 8 == 0:
                        stg = stpool.tile([128, 8, 16], F32, tag="stg")
                    nc.vector.tensor_add(stg[:, t % 8, :], sT[:], sh_t[:])
                    if t % 8 == 7:
                        nc.sync.dma_start(
                            out=dec_d[t - 7:t + 1].rearrange("t p j -> p t j"),
                            in_=stg[:])
    return nc


def host_prep(content, sentiment, hiddens, target, embed,
              W_enc, b_enc, W_prev, b_prev, W_att,
              Wi_g, bi_g, Wh_g, bh_g, Wc_g, bc_g,
              Wi, bi, Wh, bh, Wc, bc, core):
    """Build the per-core input map (batches 4*core .. 4*core+3)."""
    import numpy as np
    import ml_dtypes
    bs = slice(4 * core, 4 * core + 4)
    hid = hiddens[bs]                                    # [4,128,1024]
    hidT = np.ascontiguousarray(hid.transpose(0, 2, 1).reshape(4, 8, 128, 128))
    ws = np.concatenate([W_prev, Wh_g[:, :512], Wh], axis=1)      # [512,1536]
    ws = np.ascontiguousarray(ws.reshape(4, 128, 1536))
    wcm = np.concatenate([Wc_g[:, :512], Wc], axis=1)             # [1024,1024]
    wcm = np.ascontiguousarray(wcm.reshape(8, 128, 1024))
    wenc = np.ascontiguousarray(W_enc.reshape(8, 128, 512))
    wx = np.concatenate([Wi_g[:, :512], Wi], axis=1)              # [256,1024]
    wx = np.ascontiguousarray(wx.reshape(2, 128, 1024))
    bx = (np.concatenate([bi_g[:512] + bh_g[:512] + bc_g[:512], bi + bh + bc])
          .reshape(1, 1024))
    bebp = np.ascontiguousarray((b_enc + b_prev).reshape(4, 128).T)
    watt = np.ascontiguousarray(
        W_att[:, 0].reshape(4, 128).T).astype(ml_dtypes.bfloat16)
    s0 = np.zeros((128, 16), np.float32)
    sh = np.zeros((128, 16), np.float32)
    for b in range(4):
        for kc in range(4):
            s0[:, 4 * kc + b] = content[4 * core + b, kc * 128:(kc + 1) * 128]
            sh[:, 4 * kc + b] = 0.5 * sentiment[4 * core + b, kc * 128:(kc + 1) * 128]
    idx = np.zeros((128, 2), np.int32)
    for th in range(2):
        for tm in range(32):
            for b in range(4):
                idx[tm * 4 + b, th] = target[4 * core + b, th * 32 + tm]
    eye = np.eye(128, dtype=np.float32)
    return dict(
        hidT=hidT.astype(np.float32), ws=ws.astype(np.float32),
        wc=wcm.astype(np.float32), wenc=wenc.astype(np.float32),
        wx=wx.astype(np.float32), bx=bx.astype(np.float32),
        bebp=bebp.astype(np.float32), watt=watt,
        s0=s0, sh=sh, idx=idx, eye=eye, eyer=eye.copy(),
        onesr=np.ones((1, 128), np.float32), z16=np.zeros((128, 16), np.float32),
        embed=embed.astype(np.float32),
    )


def dec_from_out(decT):
    """decT [64,128,16] -> A_local [4, 64, 512] (dec_h + 0.5*sent)."""
    import numpy as np
    out = np.zeros((4, 64, 512), np.float32)
    for b in range(4):
        for kc in range(4):
            out[b, :, kc * 128:(kc + 1) * 128] = decT[:, :, 4 * kc + b]
    return out


# ---------------- projection (launch B) ----------------



KC = 4          # contraction chunks (H=512)
MV = 32         # vocab 128-blocks per shard (4096 padded)
NT = 4          # BT=2048 -> 4 chunks of 512
BT = 2048
F32R = mybir.dt.float32r
F32 = mybir.dt.float32
FP8 = mybir.dt.float8e4


def build_proj():
    """3-term fp8e4m3 DoubleRow split: A8@W8 + (A8/16)@(16*Wlo) + Alo@W8.
    Each term contracts K=256/instr at 0.5 cyc/row; bf16 output."""
    nc = bass.Bass()
    at_d = nc.dram_tensor("at8", [3, 2, 128, 2, BT], FP8, kind="ExternalInput")
    wo_d = nc.dram_tensor("wo8", [2, 2, 128, 2, MV * 128], FP8,
                          kind="ExternalInput")
    bo_d = nc.dram_tensor("bo", [128, MV], F32, kind="ExternalInput")
    out_d = nc.dram_tensor("outT", [MV * 128, BT], BF16, kind="ExternalOutput")
    outv = out_d.rearrange("(m p) n -> m p n", p=128)
    TERMS = [(0, 0), (1, 1), (2, 0)]   # (activation variant, weight variant)

    with tile.TileContext(nc) as tc:
        with (
            nc.allow_low_precision(reason="fp8 split validated vs reference"),
            tc.tile_pool(name="weights", bufs=1) as wpool,
            tc.tile_pool(name="outs", bufs=6) as opool,
            tc.tile_pool(name="psum", bufs=8, space="PSUM") as ppool,
        ):
            at_t = wpool.tile([128, 3, 2, 2, BT], FP8)
            wo_t = wpool.tile([128, 2, 2, 2, MV * 128], FP8)
            bo_t = wpool.tile([128, MV], F32)
            nc.sync.dma_start(out=bo_t[:], in_=bo_d[:])
            for v in range(3):
                for kc2 in range(2):
                    for i in range(2):
                        nc.sync.dma_start(out=at_t[:, v, kc2, i, :],
                                          in_=at_d[v, kc2, :, i, :])
            for w in range(2):
                for kc2 in range(2):
                    for i in range(2):
                        for mg in range(2):
                            sl = slice(mg * 2048, (mg + 1) * 2048)
                            nc.sync.dma_start(out=wo_t[:, w, kc2, i, sl],
                                              in_=wo_d[w, kc2, :, i, sl])

            for m in range(MV):
                for nt in range(NT):
                    ps = ppool.tile([128, 512], F32)
                    n_mm = 0
                    for v, w in TERMS:
                        for kc2 in range(2):
                            n_mm += 1
                            nc.tensor.matmul(
                                ps[:],
                                wo_t[:, w, kc2, :, m * 128:(m + 1) * 128],
                                at_t[:, v, kc2, :, nt * 512:(nt + 1) * 512],
                                start=(n_mm == 1),
                                stop=(n_mm == 6),
                                perf_mode=mybir.MatmulPerfMode.DoubleRow,
                            )
                    ot = opool.tile([128, 512], BF16)
                    nc.vector.tensor_scalar_add(ot[:], ps[:], bo_t[:, m:m + 1])
                    nc.sync.dma_start(
                        out=outv[m, :, nt * 512:(nt + 1) * 512], in_=ot[:]
                    )
    return nc


# ---------------- orchestration ----------------

_B, _T, _H, _V = 32, 64, 512, 32000
_VS = _V // 8          # vocab shard
_VP = 4096             # padded shard

_cache = {}
LAST_PERF = {}


def _trace_flag():
    import os
    return bool(int(os.environ.get("BASS_KERNEL_TRACE", "0")))


def _get_progs():
    if "rec" not in _cache:
        _apply_walrus_patch()
        _cache["rec"] = build_rec()
        _cache["proj"] = build_proj()
    return _cache["rec"], _cache["proj"]


def kernel(content, sentiment, hiddens, target, embed,
           W_enc, b_enc, W_prev, b_prev, W_att, b_att,
           Wi_g, bi_g, Wh_g, bh_g, Wc_g, bc_g,
           Wi, bi, Wh, bh, Wc, bc, W_out, b_out):
    from concourse.bass_utils import run_bass_kernel_spmd

    content = np.asarray(content, np.float32)
    sentiment = np.asarray(sentiment, np.float32)
    hiddens = np.asarray(hiddens, np.float32)
    target = np.asarray(target, np.int32)
    embed = np.asarray(embed, np.float32)

    rec_nc, proj_nc = _get_progs()
    trace = _trace_flag()

    in_maps_a = [
        host_prep(content, sentiment, hiddens, target, embed,
                  np.asarray(W_enc, np.float32), np.asarray(b_enc, np.float32),
                  np.asarray(W_prev, np.float32), np.asarray(b_prev, np.float32),
                  np.asarray(W_att, np.float32),
                  np.asarray(Wi_g, np.float32), np.asarray(bi_g, np.float32),
                  np.asarray(Wh_g, np.float32), np.asarray(bh_g, np.float32),
                  np.asarray(Wc_g, np.float32), np.asarray(bc_g, np.float32),
                  np.asarray(Wi, np.float32), np.asarray(bi, np.float32),
                  np.asarray(Wh, np.float32), np.asarray(bh, np.float32),
                  np.asarray(Wc, np.float32), np.asarray(bc, np.float32), core)
        for core in range(8)
    ]
    res_a = run_bass_kernel_spmd(rec_nc, in_maps_a, core_ids=list(range(8)),
                                 trace=trace)
    A = np.empty((_B, _T, _H), np.float32)
    for core in range(8):
        A[4 * core:4 * core + 4] = dec_from_out(res_a.results[core]["decT"])

    F8 = ml_dtypes.float8_e4m3fn

    def _drpack(M):   # [512, N] f32 -> DoubleRow fp8 pack [2, 128, 2, N]
        return np.ascontiguousarray(
            M.reshape(2, 2, 128, -1).transpose(0, 2, 1, 3)).astype(F8)

    At = A.reshape(_B * _T, _H).T.astype(np.float32)    # [512, 2048]
    A8f = At.astype(F8).astype(np.float32)
    at8 = np.stack([_drpack(At), _drpack(At / 16.0), _drpack(At - A8f)])
    W_out = np.asarray(W_out, np.float32)
    b_out = np.asarray(b_out, np.float32)
    in_maps_b = []
    for core in range(8):
        wsh = np.zeros((_H, _VP), np.float32)
        wsh[:, :_VS] = W_out[:, core * _VS:(core + 1) * _VS]
        bsh = np.zeros(_VP, np.float32)
        bsh[:_VS] = 1.5 * b_out[core * _VS:(core + 1) * _VS]
        W8f = wsh.astype(F8).astype(np.float32)
        wo8 = np.stack([_drpack(wsh), _drpack(16.0 * (wsh - W8f))])
        in_maps_b.append(dict(
            at8=at8,
            wo8=wo8,
            bo=np.ascontiguousarray(bsh.reshape(_VP // 128, 128).T),
        ))
    res_b = run_bass_kernel_spmd(proj_nc, in_maps_b, core_ids=list(range(8)),
                                 trace=trace)
    out = np.empty((_B, _T, _V), np.float32)
    for core in range(8):
        sh = res_b.results[core]["outT"][:_VS]          # [4000, 2048] bf16
        out[:, :, core * _VS:(core + 1) * _VS] = (
            sh.astype(np.float32).T.reshape(_B, _T, _VS))

    if trace:
        LAST_PERF["rec_ns"] = res_a.exec_time_ns
        LAST_PERF["proj_ns"] = res_b.exec_time_ns
    return out



# revision 14
# speedup vs baseline: 1.0577x; 1.0101x over previous
"""Trainium2 Bass kernel for nn_Decoder_80315888436037.

Two SPMD launches on 8 NeuronCores:
  A) attention+GRU recurrence, data-parallel over batch (4 batches/core),
     f32r matmuls (tf32-like PE mode).
  B) vocab projection (dec_h+0.5*sent) @ W_out + 1.5*b_out, vocab-sharded
     (4000 cols/core). Error-compensated fp8e4m3 DoubleRow matmuls
     (A8@W8 + (A8/16)@(16*Wlo) + Alo@W8, K=256/instr at 0.5 cyc/row),
     bf16 logits output upcast on host. Verified rel err 3.1e-03.
Host work between launches is only gather/reshape/transpose of activations.
"""

import numpy as np
import ml_dtypes

import concourse.bass as bass
import concourse.mybir as mybir
import concourse.tile as tile



import json

import concourse.bass_utils as _bu
import concourse.bass2jax as _b2j

_MAX_W = 1
_MAX_U = 1
_orig_compile_bir_kernel = _bu.compile_bir_kernel


def _split_sync(bir_json: bytes) -> bytes:
    m = json.loads(bir_json)
    uid = [0]

    def carrier(engine, debug, waits=None, updates=None):
        uid[0] += 1
        return {
            "debug": debug,
            "engine": engine,
            "ins": [],
            "name": f"WSPLIT-{uid[0]}",
            "opcode": "EventSemaphore",
            "outs": [],
            "sync_info": {"on_update": updates or [], "on_wait": waits or []},
        }

    changed = False
    for fn in m.get("functions", []):
        for bb in fn.get("blocks", []):
            out = []
            for inst in bb.get("instructions", []):
                si = inst.get("sync_info")
                if not si:
                    out.append(inst)
                    continue
                waits = si.get("on_wait") or []
                pre = []
                if len(waits) > _MAX_W:
                    changed = True
                    keep = waits[-_MAX_W:]
                    for w in waits[:-_MAX_W]:
                        pre.append(carrier(inst["engine"], inst.get("debug", 0), waits=[w]))
                    si["on_wait"] = keep
                out.extend(pre)
                out.append(inst)
            bb["instructions"] = out
    if not changed:
        return bir_json
    return json.dumps(m).encode()


def _patched_compile_bir_kernel(bir_json, tmpdir, neff_name="file.neff"):
    if isinstance(bir_json, str):
        bir_json = bir_json.encode()
    return _orig_compile_bir_kernel(_split_sync(bir_json), tmpdir, neff_name=neff_name)


def _apply_walrus_patch():
    _bu.compile_bir_kernel = _patched_compile_bir_kernel
    _b2j.compile_bir_kernel = _patched_compile_bir_kernel


# ---------------- recurrence (launch A) ----------------



F32R = mybir.dt.float32r
F32 = mybir.dt.float32
BF16 = mybir.dt.bfloat16
I32 = mybir.dt.int32
AF = mybir.ActivationFunctionType

T = 64
NB = 4  # batches per core


def build_rec(debug=False):
    nc = bass.Bass()
    hidT_d = nc.dram_tensor("hidT", [NB, 8, 128, 128], F32R, kind="ExternalInput")
    ws_d = nc.dram_tensor("ws", [4, 128, 1536], F32R, kind="ExternalInput")
    wc_d = nc.dram_tensor("wc", [8, 128, 1024], F32R, kind="ExternalInput")
    wenc_d = nc.dram_tensor("wenc", [8, 128, 512], F32R, kind="ExternalInput")
    wx_d = nc.dram_tensor("wx", [2, 128, 1024], F32R, kind="ExternalInput")
    bx_d = nc.dram_tensor("bx", [1, 1024], F32R, kind="ExternalInput")
    bebp_d = nc.dram_tensor("bebp", [128, 4], F32, kind="ExternalInput")
    watt_d = nc.dram_tensor("watt", [128, 4], BF16, kind="ExternalInput")
    s0_d = nc.dram_tensor("s0", [128, 16], F32R, kind="ExternalInput")
    sh_d = nc.dram_tensor("sh", [128, 16], F32, kind="ExternalInput")
    idx_d = nc.dram_tensor("idx", [128, 2], I32, kind="ExternalInput")
    eye_d = nc.dram_tensor("eye", [128, 128], F32, kind="ExternalInput")
    eyer_d = nc.dram_tensor("eyer", [128, 128], F32R, kind="ExternalInput")
    ones_d = nc.dram_tensor("onesr", [1, 128], F32R, kind="ExternalInput")
    z16_d = nc.dram_tensor("z16", [128, 16], F32R, kind="ExternalInput")
    embed_d = nc.dram_tensor("embed", [32000, 256], F32, kind="ExternalInput")
    dec_d = nc.dram_tensor("decT", [T, 128, 16], F32, kind="ExternalOutput")
    if debug:
        dbg_enc = nc.dram_tensor("dbg_enc", [128, 2048], F32, kind="ExternalOutput")
        dbg_pc = nc.dram_tensor("dbg_pc", [128, 4, 1024], F32, kind="ExternalOutput")
        dbg_xp = nc.dram_tensor("dbg_xp", [128, 2, 1024], F32, kind="ExternalOutput")
        dbg_emb = nc.dram_tensor("dbg_emb", [128, 2, 2, 128], F32, kind="ExternalOutput")
        dbg_q = nc.dram_tensor("dbg_q", [128, 16], F32, kind="ExternalOutput")
        dbg_sc = nc.dram_tensor("dbg_sc", [1, 512], F32, kind="ExternalOutput")
        dbg_ar = nc.dram_tensor("dbg_ar", [1, 512], F32, kind="ExternalOutput")
        dbg_atb = nc.dram_tensor("dbg_atb", [128, 16], F32, kind="ExternalOutput")
        dbg_gates = nc.dram_tensor("dbg_gates", [4, 1536], F32, kind="ExternalOutput")

    with tile.TileContext(nc) as tc:
        with (
            nc.allow_low_precision(reason="float32r tiles carry full fp32 bits"),
            tc.tile_pool(name="const", bufs=1) as cpool,
            tc.tile_pool(name="state", bufs=2) as spool,
            tc.tile_pool(name="work", bufs=2) as wpool,
            tc.tile_pool(name="stage", bufs=2) as stpool,
        ):
            # ---- resident constants/weights ----
            ws_t = cpool.tile([128, 4, 1536], F32R)
            wc_t = cpool.tile([128, 8, 1024], F32R)
            wenc_t = cpool.tile([128, 8, 512], F32R)
            wx_t = cpool.tile([128, 2, 1024], F32R)
            bx_t = cpool.tile([1, 1024], F32R)
            bebp_t = cpool.tile([128, 4], F32)
            watt_t = cpool.tile([128, 4], BF16)
            sh_t = cpool.tile([128, 16], F32)
            idx_t = cpool.tile([128, 2], I32)
            eye_t = cpool.tile([128, 128], F32)
            eyer_t = cpool.tile([128, 128], F32R)
            ones_t = cpool.tile([1, 128], F32R)
            # prologue-critical loads first (gather idx, X-pack, enc/PC
            # weights); step-only weights (ws, watt, ...) issued last so the
            # DMA queue doesn't delay the first prologue matmuls
            nc.sync.dma_start(out=idx_t[:], in_=idx_d[:])
            nc.sync.dma_start(out=eye_t[:], in_=eye_d[:])
            for kc in range(2):
                nc.sync.dma_start(out=wx_t[:, kc, :], in_=wx_d[kc])
            nc.sync.dma_start(out=bx_t[:], in_=bx_d[:])
            nc.sync.dma_start(out=ones_t[:], in_=ones_d[:])
            for kc in range(8):
                nc.sync.dma_start(out=wenc_t[:, kc, :], in_=wenc_d[kc])
                nc.sync.dma_start(out=wc_t[:, kc, :], in_=wc_d[kc])
            nc.sync.dma_start(out=bebp_t[:], in_=bebp_d[:])

            encT_t = cpool.tile([128, 2048], F32)     # (b, hc, s)
            pc_t = cpool.tile([128, 4, 1024], F32R)   # [s, b, n]
            embT_t = cpool.tile([128, 2, 2, 128], F32R)
            xp_t = cpool.tile([128, 2, 1024], F32R)

            # ---- prologue: gather, X_pack, encT, P_c ----
            with (
                tc.tile_pool(name="pro", bufs=2) as propool,
                tc.tile_pool(name="props", bufs=1, space="PSUM") as propspool,
            ):
                for th in range(2):
                    erows = propool.tile([128, 256], F32, tag="erows")
                    nc.gpsimd.indirect_dma_start(
                        out=erows[:],
                        out_offset=None,
                        in_=embed_d[:],
                        in_offset=bass.IndirectOffsetOnAxis(
                            ap=idx_t[:, th:th + 1], axis=0),
                    )
                    for kc in range(2):
                        ptr = propspool.tile([128, 128], F32, tag="ptr")
                        nc.tensor.transpose(
                            out=ptr[:], in_=erows[:, kc * 128:(kc + 1) * 128],
                            identity=eye_t[:],
                        )
                        nc.vector.tensor_copy(out=embT_t[:, kc, th, :], in_=ptr[:])

                for th in range(2):
                    psx = propspool.tile([128, 1024], F32, tag="psx")
                    for n2 in range(2):
                        sl = slice(n2 * 512, (n2 + 1) * 512)
                        for kc in range(2):
                            nc.tensor.matmul(
                                psx[:, sl], embT_t[:, kc, th, :], wx_t[:, kc, sl],
                                start=(kc == 0), stop=False,
                            )
                        nc.tensor.matmul(
                            psx[:, sl], ones_t[:], bx_t[:, sl],
                            start=False, stop=True,
                        )
                    nc.vector.tensor_copy(out=xp_t[:, th, :], in_=psx[:])

                for b in range(NB):
                    hb = propool.tile([128, 8, 128], F32R, tag="hb")
                    for kc in range(8):
                        nc.sync.dma_start(out=hb[:, kc, :], in_=hidT_d[b, kc])
                    for hc in range(4):
                        pse = propspool.tile([128, 128], F32, tag="pse")
                        for kc in range(8):
                            nc.tensor.matmul(
                                pse[:], wenc_t[:, kc, hc * 128:(hc + 1) * 128],
                                hb[:, kc, :],
                                start=(kc == 0), stop=(kc == 7),
                            )
                        nc.vector.tensor_scalar_add(
                            encT_t[:, b * 512 + hc * 128: b * 512 + (hc + 1) * 128],
                            pse[:], bebp_t[:, hc:hc + 1],
                        )
                    psp = propspool.tile([128, 1024], F32, tag="psp")
                    for n2 in range(2):
                        sl = slice(n2 * 512, (n2 + 1) * 512)
                        for kc in range(8):
                            nc.tensor.matmul(
                                psp[:, sl], hb[:, kc, :], wc_t[:, kc, sl],
                                start=(kc == 0), stop=(kc == 7),
                            )
                    nc.vector.tensor_copy(out=pc_t[:, b, :], in_=psp[:])

            # step-only weights: queued after the prologue's hidT loads
            for kc in range(4):
                nc.sync.dma_start(out=ws_t[:, kc, :], in_=ws_d[kc])
            nc.sync.dma_start(out=watt_t[:], in_=watt_d[:])
            nc.sync.dma_start(out=sh_t[:], in_=sh_d[:])
            nc.sync.dma_start(out=eyer_t[:], in_=eyer_d[:])

            # ---- state ----
            sT = spool.tile([128, 16], F32R, tag="sT")
            nc.sync.dma_start(out=sT[:], in_=s0_d[:])
            atb = cpool.tile([128, 16], F32R)  # block-diag alphaT: col 5b = alpha_b
            nc.sync.dma_start(out=atb[:], in_=z16_d[:])
            if debug:
                nc.sync.dma_start(out=dbg_enc[:], in_=encT_t[:])
                nc.gpsimd.dma_start(out=dbg_pc[:], in_=pc_t[:])
                nc.gpsimd.dma_start(out=dbg_xp[:], in_=xp_t[:])
                nc.gpsimd.dma_start(out=dbg_emb[:], in_=embT_t[:])

            # ---- recurrence ----
            with (
                tc.tile_pool(name="psA", bufs=3, space="PSUM") as psA,
                tc.tile_pool(name="psB", bufs=2, space="PSUM") as psB,
            ):
                stg = None
                for t in range(T):
                    tm, th = t % 32, t // 32
                    # 1. q matmuls
                    psc = psA.tile([128, 1024], F32, tag="psc")
                    psq = psB.tile([128, 16], F32, tag="misc")
                    for hc in range(4):
                        for kc in range(4):
                            nc.tensor.matmul(
                                psq[:, 4 * hc:4 * hc + 4],
                                ws_t[:, kc, hc * 128:(hc + 1) * 128],
                                sT[:, 4 * kc:4 * kc + 4],
                                start=(kc == 0), stop=(kc == 3),
                                skip_group_check=True,
                            )
                    qT = wpool.tile([128, 16], F32, tag="qT")
                    nc.vector.tensor_copy(out=qT[:], in_=psq[:])
                    # 2. hT = tanh(encT + qT)
                    hT = wpool.tile([128, 2048], BF16, tag="hT")
                    hpre = wpool.tile([128, 2048], F32, tag="hpre")
                    for b in range(NB):
                        for hc in range(4):
                            sl = slice(b * 512 + hc * 128, b * 512 + (hc + 1) * 128)
                            nc.vector.tensor_scalar_add(
                                hpre[:, sl], encT_t[:, sl],
                                qT[:, 4 * hc + b:4 * hc + b + 1],
                            )
                        nc.scalar.activation(
                            hT[:, b * 512:(b + 1) * 512],
                            hpre[:, b * 512:(b + 1) * 512], AF.Tanh,
                        )
                    # 3. score + alpha
                    pssc = psB.tile([1, 512], F32, tag="misc")
                    for b in range(NB):
                        for hc in range(4):
                            nc.tensor.matmul(
                                pssc[0:1, b * 128:(b + 1) * 128],
                                watt_t[:, hc:hc + 1],
                                hT[:, b * 512 + hc * 128: b * 512 + (hc + 1) * 128],
                                start=(hc == 0), stop=(hc == 3),
                                skip_group_check=True,
                            )
                    arow = wpool.tile([1, 512], F32, tag="arow")
                    sums = wpool.tile([1, 4], F32, tag="sums")
                    for b in range(NB):
                        nc.scalar.activation(
                            arow[0:1, b * 128:(b + 1) * 128],
                            pssc[0:1, b * 128:(b + 1) * 128],
                            AF.Exp, accum_out=sums[0:1, b:b + 1],
                        )
                    if debug and t == 0:
                        scrow = wpool.tile([1, 512], F32, tag="scrow")
                        nc.vector.tensor_copy(out=scrow[:], in_=pssc[:])
                        nc.sync.dma_start(out=dbg_sc[:], in_=scrow[:])
                        nc.sync.dma_start(out=dbg_ar[:], in_=arow[:])
                    rsum = wpool.tile([1, 4], F32R, tag="rsum")
                    nc.vector.reciprocal(out=rsum[:], in_=sums[:])
                    psrs = psB.tile([128, 4], F32, tag="misc")
                    nc.tensor.matmul(psrs[:], ones_t[:], rsum[:])
                    rsb = wpool.tile([128, 4], F32, tag="rsb")
                    nc.vector.tensor_copy(out=rsb[:], in_=psrs[:])
                    psa = psB.tile([128, 4], F32, tag="misc")
                    for b in range(NB):
                        nc.tensor.transpose(
                            out=psa[:, b:b + 1],
                            in_=arow[0:1, b * 128:(b + 1) * 128],
                            identity=eye_t[0:1, 0:1])
                    for b in range(NB):
                        nc.vector.tensor_mul(
                            atb[:, 5 * b:5 * b + 1], psa[:, b:b + 1],
                            rsb[:, b:b + 1])
                    if debug and t == 0:
                        nc.gpsimd.dma_start(out=dbg_atb[:], in_=atb[:])
                    # 4. r/st matmuls
                    for g in (1, 2):
                        o = psc[0:4, (g - 1) * 512:g * 512]
                        xsl = slice((g - 1) * 512, g * 512)
                        for kc in range(4):
                            nc.tensor.matmul(
                                o, sT[:, 4 * kc:4 * kc + 4],
                                ws_t[:, kc, g * 512:(g + 1) * 512],
                                start=(kc == 0), stop=False,
                                skip_group_check=True,
                            )
                        nc.tensor.matmul(
                            o, eyer_t[:, 4 * tm:4 * tm + 4], xp_t[:, th, xsl],
                            start=False, stop=False,
                            skip_group_check=True,
                        )
                    for g in (1, 2):
                        o = psc[0:4, (g - 1) * 512:g * 512]
                        xsl = slice((g - 1) * 512, g * 512)
                        for bp in range(NB):
                            nc.tensor.matmul(
                                o, atb[:, 4 * bp:4 * bp + 4], pc_t[:, bp, xsl],
                                start=False, stop=(bp == NB - 1),
                                skip_group_check=True,
                            )
                    # 5. gates
                    trows = wpool.tile([4, 512], F32, tag="trows")
                    strows = wpool.tile([4, 512], F32, tag="strows")
                    nc.scalar.activation(trows[:], psc[0:4, 0:512], AF.Tanh, scale=0.5)
                    nc.scalar.activation(strows[:], psc[0:4, 512:1024], AF.Tanh)
                    psg = psB.tile([128, 32], F32, tag="misc")
                    for hc in range(4):
                        nc.tensor.transpose(
                            out=psg[:, 4 * hc:4 * hc + 4],
                            in_=trows[:, hc * 128:(hc + 1) * 128],
                            identity=eye_t[0:4, 0:4],
                        )
                        nc.tensor.transpose(
                            out=psg[:, 16 + 4 * hc:16 + 4 * hc + 4],
                            in_=strows[:, hc * 128:(hc + 1) * 128],
                            identity=eye_t[0:4, 0:4],
                        )
                    # 6. update: r = 0.5*tau + 0.5 ; s' = s + r*(st - s)
                    r_t = wpool.tile([128, 16], F32, tag="r_t")
                    nc.vector.tensor_scalar(
                        r_t[:], psg[:, 0:16], 0.5, 0.5,
                        mybir.AluOpType.mult, mybir.AluOpType.add)
                    d_t = wpool.tile([128, 16], F32, tag="d_t")
                    nc.vector.tensor_sub(d_t[:], psg[:, 16:32], sT[:])
                    p_t = wpool.tile([128, 16], F32, tag="p_t")
                    nc.vector.tensor_mul(p_t[:], r_t[:], d_t[:])
                    sTn = spool.tile([128, 16], F32R, tag="sT")
                    nc.vector.tensor_add(sTn[:], sT[:], p_t[:])
                    sT = sTn
                    # stage output (+0.5*sentiment), DMA every 8 steps
                    if t % 8 == 0:
                        stg = stpool.tile([128, 8, 16], F32, tag="stg")
                    nc.vector.tensor_add(stg[:, t % 8, :], sT[:], sh_t[:])
                    if t % 8 == 7:
                        nc.sync.dma_start(
                            out=dec_d[t - 7:t + 1].rearrange("t p j -> p t j"),
                            in_=stg[:])
    return nc


def host_prep(content, sentiment, hiddens, target, embed,
              W_enc, b_enc, W_prev, b_prev, W_att,
              Wi_g, bi_g, Wh_g, bh_g, Wc_g, bc_g,
              Wi, bi, Wh, bh, Wc, bc, core):
    """Build the per-core input map (batches 4*core .. 4*core+3)."""
    import numpy as np
    import ml_dtypes
    bs = slice(4 * core, 4 * core + 4)
    hid = hiddens[bs]                                    # [4,128,1024]
    hidT = np.ascontiguousarray(hid.transpose(0, 2, 1).reshape(4, 8, 128, 128))
    ws = np.concatenate([W_prev, Wh_g[:, :512], Wh], axis=1)      # [512,1536]
    ws = np.ascontiguousarray(ws.reshape(4, 128, 1536))
    wcm = np.concatenate([Wc_g[:, :512], Wc], axis=1)             # [1024,1024]
    wcm = np.ascontiguousarray(wcm.reshape(8, 128, 1024))
    wenc = np.ascontiguousarray(W_enc.reshape(8, 128, 512))
    wx = np.concatenate([Wi_g[:, :512], Wi], axis=1)              # [256,1024]
    wx = np.ascontiguousarray(wx.reshape(2, 128, 1024))
    bx = (np.concatenate([bi_g[:512] + bh_g[:512] + bc_g[:512], bi + bh + bc])
          .reshape(1, 1024))
    bebp = np.ascontiguousarray((b_enc + b_prev).reshape(4, 128).T)
    watt = np.ascontiguousarray(
        W_att[:, 0].reshape(4, 128).T).astype(ml_dtypes.bfloat16)
    s0 = np.zeros((128, 16), np.float32)
    sh = np.zeros((128, 16), np.float32)
    for b in range(4):
        for kc in range(4):
            s0[:, 4 * kc + b] = content[4 * core + b, kc * 128:(kc + 1) * 128]
            sh[:, 4 * kc + b] = 0.5 * sentiment[4 * core + b, kc * 128:(kc + 1) * 128]
    idx = np.zeros((128, 2), np.int32)
    for th in range(2):
        for tm in range(32):
            for b in range(4):
                idx[tm * 4 + b, th] = target[4 * core + b, th * 32 + tm]
    eye = np.eye(128, dtype=np.float32)
    return dict(
        hidT=hidT.astype(np.float32), ws=ws.astype(np.float32),
        wc=wcm.astype(np.float32), wenc=wenc.astype(np.float32),
        wx=wx.astype(np.float32), bx=bx.astype(np.float32),
        bebp=bebp.astype(np.float32), watt=watt,
        s0=s0, sh=sh, idx=idx, eye=eye, eyer=eye.copy(),
        onesr=np.ones((1, 128), np.float32), z16=np.zeros((128, 16), np.float32),
        embed=embed.astype(np.float32),
    )


def dec_from_out(decT):
    """decT [64,128,16] -> A_local [4, 64, 512] (dec_h + 0.5*sent)."""
    import numpy as np
    out = np.zeros((4, 64, 512), np.float32)
    for b in range(4):
        for kc in range(4):
            out[b, :, kc * 128:(kc + 1) * 128] = decT[:, :, 4 * kc + b]
    return out


# ---------------- projection (launch B) ----------------



KC = 4          # contraction chunks (H=512)
MV = 32         # vocab 128-blocks per shard (4096 padded)
NT = 4          # BT=2048 -> 4 chunks of 512
BT = 2048
F32R = mybir.dt.float32r
F32 = mybir.dt.float32
FP8 = mybir.dt.float8e4


def build_proj():
    """3-term fp8e4m3 DoubleRow split: A8@W8 + (A8/16)@(16*Wlo) + Alo@W8.
    Each term contracts K=256/instr at 0.5 cyc/row; bf16 output."""
    nc = bass.Bass()
    at_d = nc.dram_tensor("at8", [3, 2, 128, 2, BT], FP8, kind="ExternalInput")
    wo_d = nc.dram_tensor("wo8", [2, 2, 128, 2, MV * 128], FP8,
                          kind="ExternalInput")
    bo_d = nc.dram_tensor("bo", [128, MV], F32, kind="ExternalInput")
    out_d = nc.dram_tensor("outT", [MV * 128, BT], BF16, kind="ExternalOutput")
    outv = out_d.rearrange("(m p) n -> m p n", p=128)
    TERMS = [(0, 0), (1, 1), (2, 0)]   # (activation variant, weight variant)

    with tile.TileContext(nc) as tc:
        with (
            nc.allow_low_precision(reason="fp8 split validated vs reference"),
            tc.tile_pool(name="weights", bufs=1) as wpool,
            tc.tile_pool(name="outs", bufs=6) as opool,
            tc.tile_pool(name="psum", bufs=8, space="PSUM") as ppool,
        ):
            at_t = wpool.tile([128, 3, 2, 2, BT], FP8)
            wo_t = wpool.tile([128, 2, 2, 2, MV * 128], FP8)
            bo_t = wpool.tile([128, MV], F32)
            nc.sync.dma_start(out=bo_t[:], in_=bo_d[:])
            for v in range(3):
                for kc2 in range(2):
                    for i in range(2):
                        nc.sync.dma_start(out=at_t[:, v, kc2, i, :],
                                          in_=at_d[v, kc2, :, i, :])
            for w in range(2):
                for kc2 in range(2):
                    for i in range(2):
                        for mg in range(2):
                            sl = slice(mg * 2048, (mg + 1) * 2048)
                            nc.sync.dma_start(out=wo_t[:, w, kc2, i, sl],
                                              in_=wo_d[w, kc2, :, i, sl])

            for m in range(MV):
                for nt in range(NT):
                    ps = ppool.tile([128, 512], F32)
                    n_mm = 0
                    for v, w in TERMS:
                        for kc2 in range(2):
                            n_mm += 1
                            nc.tensor.matmul(
                                ps[:],
                                wo_t[:, w, kc2, :, m * 128:(m + 1) * 128],
                                at_t[:, v, kc2, :, nt * 512:(nt + 1) * 512],
                                start=(n_mm == 1),
                                stop=(n_mm == 6),
                                perf_mode=mybir.MatmulPerfMode.DoubleRow,
                            )
                    ot = opool.tile([128, 512], BF16)
                    nc.vector.tensor_scalar_add(ot[:], ps[:], bo_t[:, m:m + 1])
                    nc.sync.dma_start(
                        out=outv[m, :, nt * 512:(nt + 1) * 512], in_=ot[:]
                    )
    return nc


# ---------------- orchestration ----------------

_B, _T, _H, _V = 32, 64, 512, 32000
_VS = _V // 8          # vocab shard
_VP = 4096             # padded shard

_cache = {}
LAST_PERF = {}


def _trace_flag():
    import os
    return bool(int(os.environ.get("BASS_KERNEL_TRACE", "0")))


def _get_progs():
    if "rec" not in _cache:
        _apply_walrus_patch()
        _cache["rec"] = build_rec()
        _cache["proj"] = build_proj()
    return _cache["rec"], _cache["proj"]


def kernel(content, sentiment, hiddens, target, embed,
           W_enc, b_enc, W_prev, b_prev, W_att, b_att,
           Wi_g, bi_g, Wh_g, bh_g, Wc_g, bc_g,
           Wi, bi, Wh, bh, Wc, bc, W_out, b_out):
    from concourse.bass_utils import run_bass_kernel_spmd

    content = np.asarray(content, np.float32)
    sentiment = np.asarray(sentiment, np.float32)
    hiddens = np.asarray(hiddens, np.float32)
    target = np.asarray(target, np.int32)
    embed = np.asarray(embed, np.float32)

    rec_nc, proj_nc = _get_progs()
    trace = _trace_flag()

    in_maps_a = [
        host_prep(content, sentiment, hiddens, target, embed,
                  np.asarray(W_enc, np.float32), np.asarray(b_enc, np.float32),
                  np.asarray(W_prev, np.float32), np.asarray(b_prev, np.float32),
                  np.asarray(W_att, np.float32),
                  np.asarray(Wi_g, np.float32), np.asarray(bi_g, np.float32),
                  np.asarray(Wh_g, np.float32), np.asarray(bh_g, np.float32),
                  np.asarray(Wc_g, np.float32), np.asarray(bc_g, np.float32),
                  np.asarray(Wi, np.float32), np.asarray(bi, np.float32),
                  np.asarray(Wh, np.float32), np.asarray(bh, np.float32),
                  np.asarray(Wc, np.float32), np.asarray(bc, np.float32), core)
        for core in range(8)
    ]
    res_a = run_bass_kernel_spmd(rec_nc, in_maps_a, core_ids=list(range(8)),
                                 trace=trace)
    A = np.empty((_B, _T, _H), np.float32)
    for core in range(8):
        A[4 * core:4 * core + 4] = dec_from_out(res_a.results[core]["decT"])

    F8 = ml_dtypes.float8_e4m3fn

    def _drpack(M):   # [512, N] f32 -> DoubleRow fp8 pack [2, 128, 2, N]
        return np.ascontiguousarray(
            M.reshape(2, 2, 128, -1).transpose(0, 2, 1, 3)).astype(F8)

    At = A.reshape(_B * _T, _H).T.astype(np.float32)    # [512, 2048]
    A8f = At.astype(F8).astype(np.float32)
    at8 = np.stack([_drpack(At), _drpack(At / 16.0), _drpack(At - A8f)])
    W_out = np.asarray(W_out, np.float32)
    b_out = np.asarray(b_out, np.float32)
    in_maps_b = []
    for core in range(8):
        wsh = np.zeros((_H, _VP), np.float32)
        wsh[:, :_VS] = W_out[:, core * _VS:(core + 1) * _VS]
        bsh = np.zeros(_VP, np.float32)
        bsh[:_VS] = 1.5 * b_out[core * _VS:(core + 1) * _VS]
        W8f = wsh.astype(F8).astype(np.float32)
        wo8 = np.stack([_drpack(wsh), _drpack(16.0 * (wsh - W8f))])
        in_maps_b.append(dict(
            at8=at8,
            wo8=wo8,
            bo=np.ascontiguousarray(bsh.reshape(_VP // 128, 128).T),
        ))
    res_b = run_bass_kernel_spmd(proj_nc, in_maps_b, core_ids=list(range(8)),
                                 trace=trace)
    out = np.empty((_B, _T, _V), np.float32)
    for core in range(8):
        sh = res_b.results[core]["outT"][:_VS]          # [4000, 2048] bf16
        out[:, :, core * _VS:(core + 1) * _VS] = (
            sh.astype(np.float32).T.reshape(_B, _T, _VS))

    if trace:
        LAST_PERF["rec_ns"] = res_a.exec_time_ns
        LAST_PERF["proj_ns"] = res_b.exec_time_ns
    return out



# revision 15
# speedup vs baseline: 1.0596x; 1.0018x over previous
"""Trainium2 Bass kernel for nn_Decoder_80315888436037.

Two SPMD launches on 8 NeuronCores:
  A) attention+GRU recurrence, data-parallel over batch (4 batches/core),
     f32r matmuls (tf32-like PE mode).
  B) vocab projection (dec_h+0.5*sent) @ W_out + 1.5*b_out, vocab-sharded
     (4000 cols/core). Error-compensated fp8e4m3 DoubleRow matmuls
     (A8@W8 + (A8/16)@(16*Wlo) + Alo@W8, K=256/instr at 0.5 cyc/row),
     bf16 logits output upcast on host. Verified rel err 3.1e-03.
Host work between launches is only gather/reshape/transpose of activations.
"""

import numpy as np
import ml_dtypes

import concourse.bass as bass
import concourse.mybir as mybir
import concourse.tile as tile



import json

import concourse.bass_utils as _bu
import concourse.bass2jax as _b2j

_MAX_W = 1
_MAX_U = 1
_orig_compile_bir_kernel = _bu.compile_bir_kernel


def _split_sync(bir_json: bytes) -> bytes:
    m = json.loads(bir_json)
    uid = [0]

    def carrier(engine, debug, waits=None, updates=None):
        uid[0] += 1
        return {
            "debug": debug,
            "engine": engine,
            "ins": [],
            "name": f"WSPLIT-{uid[0]}",
            "opcode": "EventSemaphore",
            "outs": [],
            "sync_info": {"on_update": updates or [], "on_wait": waits or []},
        }

    changed = False
    for fn in m.get("functions", []):
        for bb in fn.get("blocks", []):
            out = []
            for inst in bb.get("instructions", []):
                si = inst.get("sync_info")
                if not si:
                    out.append(inst)
                    continue
                waits = si.get("on_wait") or []
                pre = []
                if len(waits) > _MAX_W:
                    changed = True
                    keep = waits[-_MAX_W:]
                    for w in waits[:-_MAX_W]:
                        pre.append(carrier(inst["engine"], inst.get("debug", 0), waits=[w]))
                    si["on_wait"] = keep
                out.extend(pre)
                out.append(inst)
            bb["instructions"] = out
    if not changed:
        return bir_json
    return json.dumps(m).encode()


def _patched_compile_bir_kernel(bir_json, tmpdir, neff_name="file.neff"):
    if isinstance(bir_json, str):
        bir_json = bir_json.encode()
    return _orig_compile_bir_kernel(_split_sync(bir_json), tmpdir, neff_name=neff_name)


def _apply_walrus_patch():
    _bu.compile_bir_kernel = _patched_compile_bir_kernel
    _b2j.compile_bir_kernel = _patched_compile_bir_kernel


# ---------------- recurrence (launch A) ----------------



F32R = mybir.dt.float32r
F32 = mybir.dt.float32
BF16 = mybir.dt.bfloat16
I32 = mybir.dt.int32
AF = mybir.ActivationFunctionType

T = 64
NB = 4  # batches per core


def build_rec(debug=False):
    nc = bass.Bass()
    hidT_d = nc.dram_tensor("hidT", [NB, 8, 128, 128], F32R, kind="ExternalInput")
    ws_d = nc.dram_tensor("ws", [4, 128, 1536], F32R, kind="ExternalInput")
    wc_d = nc.dram_tensor("wc", [8, 128, 1024], F32R, kind="ExternalInput")
    wenc_d = nc.dram_tensor("wenc", [8, 128, 512], F32R, kind="ExternalInput")
    wx_d = nc.dram_tensor("wx", [2, 128, 1024], F32R, kind="ExternalInput")
    bx_d = nc.dram_tensor("bx", [1, 1024], F32R, kind="ExternalInput")
    bebp_d = nc.dram_tensor("bebp", [128, 4], F32, kind="ExternalInput")
    watt_d = nc.dram_tensor("watt", [128, 4], BF16, kind="ExternalInput")
    s0_d = nc.dram_tensor("s0", [128, 16], F32R, kind="ExternalInput")
    sh_d = nc.dram_tensor("sh", [128, 16], F32, kind="ExternalInput")
    idx_d = nc.dram_tensor("idx", [128, 2], I32, kind="ExternalInput")
    eye_d = nc.dram_tensor("eye", [128, 128], F32, kind="ExternalInput")
    eyer_d = nc.dram_tensor("eyer", [128, 128], F32R, kind="ExternalInput")
    ones_d = nc.dram_tensor("onesr", [1, 128], F32R, kind="ExternalInput")
    z16_d = nc.dram_tensor("z16", [128, 16], F32R, kind="ExternalInput")
    embed_d = nc.dram_tensor("embed", [32000, 256], F32, kind="ExternalInput")
    dec_d = nc.dram_tensor("decT", [T, 128, 16], F32, kind="ExternalOutput")
    if debug:
        dbg_enc = nc.dram_tensor("dbg_enc", [128, 2048], F32, kind="ExternalOutput")
        dbg_pc = nc.dram_tensor("dbg_pc", [128, 4, 1024], F32, kind="ExternalOutput")
        dbg_xp = nc.dram_tensor("dbg_xp", [128, 2, 1024], F32, kind="ExternalOutput")
        dbg_emb = nc.dram_tensor("dbg_emb", [128, 2, 2, 128], F32, kind="ExternalOutput")
        dbg_q = nc.dram_tensor("dbg_q", [128, 16], F32, kind="ExternalOutput")
        dbg_sc = nc.dram_tensor("dbg_sc", [1, 512], F32, kind="ExternalOutput")
        dbg_ar = nc.dram_tensor("dbg_ar", [1, 512], F32, kind="ExternalOutput")
        dbg_atb = nc.dram_tensor("dbg_atb", [128, 16], F32, kind="ExternalOutput")
        dbg_gates = nc.dram_tensor("dbg_gates", [4, 1536], F32, kind="ExternalOutput")

    with tile.TileContext(nc) as tc:
        with (
            nc.allow_low_precision(reason="float32r tiles carry full fp32 bits"),
            tc.tile_pool(name="const", bufs=1) as cpool,
            tc.tile_pool(name="state", bufs=2) as spool,
            tc.tile_pool(name="work", bufs=2) as wpool,
            tc.tile_pool(name="stage", bufs=2) as stpool,
        ):
            # ---- resident constants/weights ----
            ws_t = cpool.tile([128, 4, 1536], F32R)
            wc_t = cpool.tile([128, 8, 1024], F32R)
            wenc_t = cpool.tile([128, 8, 512], F32R)
            wx_t = cpool.tile([128, 2, 1024], F32R)
            bx_t = cpool.tile([1, 1024], F32R)
            bebp_t = cpool.tile([128, 4], F32)
            watt_t = cpool.tile([128, 4], BF16)
            sh_t = cpool.tile([128, 16], F32)
            idx_t = cpool.tile([128, 2], I32)
            eye_t = cpool.tile([128, 128], F32)
            eyer_t = cpool.tile([128, 128], F32R)
            ones_t = cpool.tile([1, 128], F32R)
            # prologue-critical loads first (gather idx, X-pack, enc/PC
            # weights); step-only weights (ws, watt, ...) issued last so the
            # DMA queue doesn't delay the first prologue matmuls
            nc.sync.dma_start(out=idx_t[:], in_=idx_d[:])
            nc.sync.dma_start(out=eye_t[:], in_=eye_d[:])
            for kc in range(2):
                nc.sync.dma_start(out=wx_t[:, kc, :], in_=wx_d[kc])
            nc.sync.dma_start(out=bx_t[:], in_=bx_d[:])
            nc.sync.dma_start(out=ones_t[:], in_=ones_d[:])
            hb0_t = cpool.tile([128, 8, 128], F32R)
            for kc in range(8):
                nc.sync.dma_start(out=wenc_t[:, kc, :], in_=wenc_d[kc])
                nc.sync.dma_start(out=hb0_t[:, kc, :], in_=hidT_d[0, kc])
            for kc in range(8):
                nc.sync.dma_start(out=wc_t[:, kc, :], in_=wc_d[kc])
            nc.sync.dma_start(out=bebp_t[:], in_=bebp_d[:])

            encT_t = cpool.tile([128, 2048], F32)     # (b, hc, s)
            pc_t = cpool.tile([128, 4, 1024], F32R)   # [s, b, n]
            embT_t = cpool.tile([128, 2, 2, 128], F32R)
            xp_t = cpool.tile([128, 2, 1024], F32R)

            # ---- prologue: gather, X_pack, encT, P_c ----
            with (
                tc.tile_pool(name="pro", bufs=2) as propool,
                tc.tile_pool(name="props", bufs=1, space="PSUM") as propspool,
            ):
                for th in range(2):
                    erows = propool.tile([128, 256], F32, tag="erows")
                    nc.gpsimd.indirect_dma_start(
                        out=erows[:],
                        out_offset=None,
                        in_=embed_d[:],
                        in_offset=bass.IndirectOffsetOnAxis(
                            ap=idx_t[:, th:th + 1], axis=0),
                    )
                    for kc in range(2):
                        ptr = propspool.tile([128, 128], F32, tag="ptr")
                        nc.tensor.transpose(
                            out=ptr[:], in_=erows[:, kc * 128:(kc + 1) * 128],
                            identity=eye_t[:],
                        )
                        nc.vector.tensor_copy(out=embT_t[:, kc, th, :], in_=ptr[:])

                for th in range(2):
                    psx = propspool.tile([128, 1024], F32, tag="psx")
                    for n2 in range(2):
                        sl = slice(n2 * 512, (n2 + 1) * 512)
                        for kc in range(2):
                            nc.tensor.matmul(
                                psx[:, sl], embT_t[:, kc, th, :], wx_t[:, kc, sl],
                                start=(kc == 0), stop=False,
                            )
                        nc.tensor.matmul(
                            psx[:, sl], ones_t[:], bx_t[:, sl],
                            start=False, stop=True,
                        )
                    nc.vector.tensor_copy(out=xp_t[:, th, :], in_=psx[:])

                for b in range(NB):
                    if b == 0:
                        hb = hb0_t
                    else:
                        hb = propool.tile([128, 8, 128], F32R, tag="hb")
                        for kc in range(8):
                            nc.sync.dma_start(out=hb[:, kc, :], in_=hidT_d[b, kc])
                    for hc in range(4):
                        pse = propspool.tile([128, 128], F32, tag="pse")
                        for kc in range(8):
                            nc.tensor.matmul(
                                pse[:], wenc_t[:, kc, hc * 128:(hc + 1) * 128],
                                hb[:, kc, :],
                                start=(kc == 0), stop=(kc == 7),
                            )
                        nc.vector.tensor_scalar_add(
                            encT_t[:, b * 512 + hc * 128: b * 512 + (hc + 1) * 128],
                            pse[:], bebp_t[:, hc:hc + 1],
                        )
                    psp = propspool.tile([128, 1024], F32, tag="psp")
                    for n2 in range(2):
                        sl = slice(n2 * 512, (n2 + 1) * 512)
                        for kc in range(8):
                            nc.tensor.matmul(
                                psp[:, sl], hb[:, kc, :], wc_t[:, kc, sl],
                                start=(kc == 0), stop=(kc == 7),
                            )
                    nc.vector.tensor_copy(out=pc_t[:, b, :], in_=psp[:])

            # step-only weights: queued after the prologue's hidT loads
            for kc in range(4):
                nc.sync.dma_start(out=ws_t[:, kc, :], in_=ws_d[kc])
            nc.sync.dma_start(out=watt_t[:], in_=watt_d[:])
            nc.sync.dma_start(out=sh_t[:], in_=sh_d[:])
            nc.sync.dma_start(out=eyer_t[:], in_=eyer_d[:])

            # ---- state ----
            sT = spool.tile([128, 16], F32R, tag="sT")
            nc.sync.dma_start(out=sT[:], in_=s0_d[:])
            atb = cpool.tile([128, 16], F32R)  # block-diag alphaT: col 5b = alpha_b
            nc.sync.dma_start(out=atb[:], in_=z16_d[:])
            if debug:
                nc.sync.dma_start(out=dbg_enc[:], in_=encT_t[:])
                nc.gpsimd.dma_start(out=dbg_pc[:], in_=pc_t[:])
                nc.gpsimd.dma_start(out=dbg_xp[:], in_=xp_t[:])
                nc.gpsimd.dma_start(out=dbg_emb[:], in_=embT_t[:])

            # ---- recurrence ----
            with (
                tc.tile_pool(name="psA", bufs=3, space="PSUM") as psA,
                tc.tile_pool(name="psB", bufs=2, space="PSUM") as psB,
            ):
                stg = None
                for t in range(T):
                    tm, th = t % 32, t // 32
                    # 1. q matmuls
                    psc = psA.tile([128, 1024], F32, tag="psc")
                    psq = psB.tile([128, 16], F32, tag="misc")
                    for hc in range(4):
                        for kc in range(4):
                            nc.tensor.matmul(
                                psq[:, 4 * hc:4 * hc + 4],
                                ws_t[:, kc, hc * 128:(hc + 1) * 128],
                                sT[:, 4 * kc:4 * kc + 4],
                                start=(kc == 0), stop=(kc == 3),
                                skip_group_check=True,
                            )
                    qT = wpool.tile([128, 16], F32, tag="qT")
                    nc.vector.tensor_copy(out=qT[:], in_=psq[:])
                    # 2. hT = tanh(encT + qT)
                    hT = wpool.tile([128, 2048], BF16, tag="hT")
                    hpre = wpool.tile([128, 2048], F32, tag="hpre")
                    for b in range(NB):
                        for hc in range(4):
                            sl = slice(b * 512 + hc * 128, b * 512 + (hc + 1) * 128)
                            nc.vector.tensor_scalar_add(
                                hpre[:, sl], encT_t[:, sl],
                                qT[:, 4 * hc + b:4 * hc + b + 1],
                            )
                        nc.scalar.activation(
                            hT[:, b * 512:(b + 1) * 512],
                            hpre[:, b * 512:(b + 1) * 512], AF.Tanh,
                        )
                    # 3. score + alpha
                    pssc = psB.tile([1, 512], F32, tag="misc")
                    for b in range(NB):
                        for hc in range(4):
                            nc.tensor.matmul(
                                pssc[0:1, b * 128:(b + 1) * 128],
                                watt_t[:, hc:hc + 1],
                                hT[:, b * 512 + hc * 128: b * 512 + (hc + 1) * 128],
                                start=(hc == 0), stop=(hc == 3),
                                skip_group_check=True,
                            )
                    arow = wpool.tile([1, 512], F32, tag="arow")
                    sums = wpool.tile([1, 4], F32, tag="sums")
                    for b in range(NB):
                        nc.scalar.activation(
                            arow[0:1, b * 128:(b + 1) * 128],
                            pssc[0:1, b * 128:(b + 1) * 128],
                            AF.Exp, accum_out=sums[0:1, b:b + 1],
                        )
                    if debug and t == 0:
                        scrow = wpool.tile([1, 512], F32, tag="scrow")
                        nc.vector.tensor_copy(out=scrow[:], in_=pssc[:])
                        nc.sync.dma_start(out=dbg_sc[:], in_=scrow[:])
                        nc.sync.dma_start(out=dbg_ar[:], in_=arow[:])
                    rsum = wpool.tile([1, 4], F32R, tag="rsum")
                    nc.vector.reciprocal(out=rsum[:], in_=sums[:])
                    psrs = psB.tile([128, 4], F32, tag="misc")
                    nc.tensor.matmul(psrs[:], ones_t[:], rsum[:])
                    rsb = wpool.tile([128, 4], F32, tag="rsb")
                    nc.vector.tensor_copy(out=rsb[:], in_=psrs[:])
                    psa = psB.tile([128, 4], F32, tag="misc")
                    for b in range(NB):
                        nc.tensor.transpose(
                            out=psa[:, b:b + 1],
                            in_=arow[0:1, b * 128:(b + 1) * 128],
                            identity=eye_t[0:1, 0:1])
                    for b in range(NB):
                        nc.vector.tensor_mul(
                            atb[:, 5 * b:5 * b + 1], psa[:, b:b + 1],
                            rsb[:, b:b + 1])
                    if debug and t == 0:
                        nc.gpsimd.dma_start(out=dbg_atb[:], in_=atb[:])
                    # 4. r/st matmuls
                    for g in (1, 2):
                        o = psc[0:4, (g - 1) * 512:g * 512]
                        xsl = slice((g - 1) * 512, g * 512)
                        for kc in range(4):
                            nc.tensor.matmul(
                                o, sT[:, 4 * kc:4 * kc + 4],
                                ws_t[:, kc, g * 512:(g + 1) * 512],
                                start=(kc == 0), stop=False,
                                skip_group_check=True,
                            )
                        nc.tensor.matmul(
                            o, eyer_t[:, 4 * tm:4 * tm + 4], xp_t[:, th, xsl],
                            start=False, stop=False,
                            skip_group_check=True,
                        )
                    for g in (1, 2):
                        o = psc[0:4, (g - 1) * 512:g * 512]
                        xsl = slice((g - 1) * 512, g * 512)
                        for bp in range(NB):
                            nc.tensor.matmul(
                                o, atb[:, 4 * bp:4 * bp + 4], pc_t[:, bp, xsl],
                                start=False, stop=(bp == NB - 1),
                                skip_group_check=True,
                            )
                    # 5. gates
                    trows = wpool.tile([4, 512], F32, tag="trows")
                    strows = wpool.tile([4, 512], F32, tag="strows")
                    nc.scalar.activation(trows[:], psc[0:4, 0:512], AF.Tanh, scale=0.5)
                    nc.scalar.activation(strows[:], psc[0:4, 512:1024], AF.Tanh)
                    psg = psB.tile([128, 32], F32, tag="misc")
                    for hc in range(4):
                        nc.tensor.transpose(
                            out=psg[:, 4 * hc:4 * hc + 4],
                            in_=trows[:, hc * 128:(hc + 1) * 128],
                            identity=eye_t[0:4, 0:4],
                        )
                        nc.tensor.transpose(
                            out=psg[:, 16 + 4 * hc:16 + 4 * hc + 4],
                            in_=strows[:, hc * 128:(hc + 1) * 128],
                            identity=eye_t[0:4, 0:4],
                        )
                    # 6. update: r = 0.5*tau + 0.5 ; s' = s + r*(st - s)
                    r_t = wpool.tile([128, 16], F32, tag="r_t")
                    nc.vector.tensor_scalar(
                        r_t[:], psg[:, 0:16], 0.5, 0.5,
                        mybir.AluOpType.mult, mybir.AluOpType.add)
                    d_t = wpool.tile([128, 16], F32, tag="d_t")
                    nc.vector.tensor_sub(d_t[:], psg[:, 16:32], sT[:])
                    p_t = wpool.tile([128, 16], F32, tag="p_t")
                    nc.vector.tensor_mul(p_t[:], r_t[:], d_t[:])
                    sTn = spool.tile([128, 16], F32R, tag="sT")
                    nc.vector.tensor_add(sTn[:], sT[:], p_t[:])
                    sT = sTn
                    # stage output (+0.5*sentiment), DMA every 8 steps
                    if t % 8 == 0:
                        stg = stpool.tile([128, 8, 16], F32, tag="stg")
                    nc.vector.tensor_add(stg[:, t % 8, :], sT[:], sh_t[:])
                    if t % 8 == 7:
                        nc.sync.dma_start(
                            out=dec_d[t - 7:t + 1].rearrange("t p j -> p t j"),
                            in_=stg[:])
    return nc


def host_prep(content, sentiment, hiddens, target, embed,
              W_enc, b_enc, W_prev, b_prev, W_att,
              Wi_g, bi_g, Wh_g, bh_g, Wc_g, bc_g,
              Wi, bi, Wh, bh, Wc, bc, core):
    """Build the per-core input map (batches 4*core .. 4*core+3)."""
    import numpy as np
    import ml_dtypes
    bs = slice(4 * core, 4 * core + 4)
    hid = hiddens[bs]                                    # [4,128,1024]
    hidT = np.ascontiguousarray(hid.transpose(0, 2, 1).reshape(4, 8, 128, 128))
    ws = np.concatenate([W_prev, Wh_g[:, :512], Wh], axis=1)      # [512,1536]
    ws = np.ascontiguousarray(ws.reshape(4, 128, 1536))
    wcm = np.concatenate([Wc_g[:, :512], Wc], axis=1)             # [1024,1024]
    wcm = np.ascontiguousarray(wcm.reshape(8, 128, 1024))
    wenc = np.ascontiguousarray(W_enc.reshape(8, 128, 512))
    wx = np.concatenate([Wi_g[:, :512], Wi], axis=1)              # [256,1024]
    wx = np.ascontiguousarray(wx.reshape(2, 128, 1024))
    bx = (np.concatenate([bi_g[:512] + bh_g[:512] + bc_g[:512], bi + bh + bc])
          .reshape(1, 1024))
    bebp = np.ascontiguousarray((b_enc + b_prev).reshape(4, 128).T)
    watt = np.ascontiguousarray(
        W_att[:, 0].reshape(4, 128).T).astype(ml_dtypes.bfloat16)
    s0 = np.zeros((128, 16), np.float32)
    sh = np.zeros((128, 16), np.float32)
    for b in range(4):
        for kc in range(4):
            s0[:, 4 * kc + b] = content[4 * core + b, kc * 128:(kc + 1) * 128]
            sh[:, 4 * kc + b] = 0.5 * sentiment[4 * core + b, kc * 128:(kc + 1) * 128]
    idx = np.zeros((128, 2), np.int32)
    for th in range(2):
        for tm in range(32):
            for b in range(4):
                idx[tm * 4 + b, th] = target[4 * core + b, th * 32 + tm]
    eye = np.eye(128, dtype=np.float32)
    return dict(
        hidT=hidT.astype(np.float32), ws=ws.astype(np.float32),
        wc=wcm.astype(np.float32), wenc=wenc.astype(np.float32),
        wx=wx.astype(np.float32), bx=bx.astype(np.float32),
        bebp=bebp.astype(np.float32), watt=watt,
        s0=s0, sh=sh, idx=idx, eye=eye, eyer=eye.copy(),
        onesr=np.ones((1, 128), np.float32), z16=np.zeros((128, 16), np.float32),
        embed=embed.astype(np.float32),
    )


def dec_from_out(decT):
    """decT [64,128,16] -> A_local [4, 64, 512] (dec_h + 0.5*sent)."""
    import numpy as np
    out = np.zeros((4, 64, 512), np.float32)
    for b in range(4):
        for kc in range(4):
            out[b, :, kc * 128:(kc + 1) * 128] = decT[:, :, 4 * kc + b]
    return out


# ---------------- projection (launch B) ----------------



KC = 4          # contraction chunks (H=512)
MV = 32         # vocab 128-blocks per shard (4096 padded)
NT = 4          # BT=2048 -> 4 chunks of 512
BT = 2048
F32R = mybir.dt.float32r
F32 = mybir.dt.float32
FP8 = mybir.dt.float8e4


def build_proj():
    """3-term fp8e4m3 DoubleRow split: A8@W8 + (A8/16)@(16*Wlo) + Alo@W8.
    Each term contracts K=256/instr at 0.5 cyc/row; bf16 output."""
    nc = bass.Bass()
    at_d = nc.dram_tensor("at8", [3, 2, 128, 2, BT], FP8, kind="ExternalInput")
    wo_d = nc.dram_tensor("wo8", [2, 2, 128, 2, MV * 128], FP8,
                          kind="ExternalInput")
    bo_d = nc.dram_tensor("bo", [128, MV], F32, kind="ExternalInput")
    out_d = nc.dram_tensor("outT", [MV * 128, BT], BF16, kind="ExternalOutput")
    outv = out_d.rearrange("(m p) n -> m p n", p=128)
    TERMS = [(0, 0), (1, 1), (2, 0)]   # (activation variant, weight variant)

    with tile.TileContext(nc) as tc:
        with (
            nc.allow_low_precision(reason="fp8 split validated vs reference"),
            tc.tile_pool(name="weights", bufs=1) as wpool,
            tc.tile_pool(name="outs", bufs=6) as opool,
            tc.tile_pool(name="psum", bufs=8, space="PSUM") as ppool,
        ):
            at_t = wpool.tile([128, 3, 2, 2, BT], FP8)
            wo_t = wpool.tile([128, 2, 2, 2, MV * 128], FP8)
            bo_t = wpool.tile([128, MV], F32)
            nc.sync.dma_start(out=bo_t[:], in_=bo_d[:])
            for v in range(3):
                for kc2 in range(2):
                    for i in range(2):
                        nc.sync.dma_start(out=at_t[:, v, kc2, i, :],
                                          in_=at_d[v, kc2, :, i, :])
            for w in range(2):
                for kc2 in range(2):
                    for i in range(2):
                        for mg in range(2):
                            sl = slice(mg * 2048, (mg + 1) * 2048)
                            nc.sync.dma_start(out=wo_t[:, w, kc2, i, sl],
                                              in_=wo_d[w, kc2, :, i, sl])

            for m in range(MV):
                for nt in range(NT):
                    ps = ppool.tile([128, 512], F32)
                    n_mm = 0
                    for v, w in TERMS:
                        for kc2 in range(2):
                            n_mm += 1
                            nc.tensor.matmul(
                                ps[:],
                                wo_t[:, w, kc2, :, m * 128:(m + 1) * 128],
                                at_t[:, v, kc2, :, nt * 512:(nt + 1) * 512],
                                start=(n_mm == 1),
                                stop=(n_mm == 6),
                                perf_mode=mybir.MatmulPerfMode.DoubleRow,
                            )
                    ot = opool.tile([128, 512], BF16)
                    nc.vector.tensor_scalar_add(ot[:], ps[:], bo_t[:, m:m + 1])
                    nc.sync.dma_start(
                        out=outv[m, :, nt * 512:(nt + 1) * 512], in_=ot[:]
                    )
    return nc


# ---------------- orchestration ----------------

_B, _T, _H, _V = 32, 64, 512, 32000
_VS = _V // 8          # vocab shard
_VP = 4096             # padded shard

_cache = {}
LAST_PERF = {}


def _trace_flag():
    import os
    return bool(int(os.environ.get("BASS_KERNEL_TRACE", "0")))


def _get_progs():
    if "rec" not in _cache:
        _apply_walrus_patch()
        _cache["rec"] = build_rec()
        _cache["proj"] = build_proj()
    return _cache["rec"], _cache["proj"]


def kernel(content, sentiment, hiddens, target, embed,
           W_enc, b_enc, W_prev, b_prev, W_att, b_att,
           Wi_g, bi_g, Wh_g, bh_g, Wc_g, bc_g,
           Wi, bi, Wh, bh, Wc, bc, W_out, b_out):
    from concourse.bass_utils import run_bass_kernel_spmd

    content = np.asarray(content, np.float32)
    sentiment = np.asarray(sentiment, np.float32)
    hiddens = np.asarray(hiddens, np.float32)
    target = np.asarray(target, np.int32)
    embed = np.asarray(embed, np.float32)

    rec_nc, proj_nc = _get_progs()
    trace = _trace_flag()

    in_maps_a = [
        host_prep(content, sentiment, hiddens, target, embed,
                  np.asarray(W_enc, np.float32), np.asarray(b_enc, np.float32),
                  np.asarray(W_prev, np.float32), np.asarray(b_prev, np.float32),
                  np.asarray(W_att, np.float32),
                  np.asarray(Wi_g, np.float32), np.asarray(bi_g, np.float32),
                  np.asarray(Wh_g, np.float32), np.asarray(bh_g, np.float32),
                  np.asarray(Wc_g, np.float32), np.asarray(bc_g, np.float32),
                  np.asarray(Wi, np.float32), np.asarray(bi, np.float32),
                  np.asarray(Wh, np.float32), np.asarray(bh, np.float32),
                  np.asarray(Wc, np.float32), np.asarray(bc, np.float32), core)
        for core in range(8)
    ]
    res_a = run_bass_kernel_spmd(rec_nc, in_maps_a, core_ids=list(range(8)),
                                 trace=trace)
    A = np.empty((_B, _T, _H), np.float32)
    for core in range(8):
        A[4 * core:4 * core + 4] = dec_from_out(res_a.results[core]["decT"])

    F8 = ml_dtypes.float8_e4m3fn

    def _drpack(M):   # [512, N] f32 -> DoubleRow fp8 pack [2, 128, 2, N]
        return np.ascontiguousarray(
            M.reshape(2, 2, 128, -1).transpose(0, 2, 1, 3)).astype(F8)

    At = A.reshape(_B * _T, _H).T.astype(np.float32)    # [512, 2048]
    A8f = At.astype(F8).astype(np.float32)
    at8 = np.stack([_drpack(At), _drpack(At / 16.0), _drpack(At - A8f)])
    W_out = np.asarray(W_out, np.float32)
    b_out = np.asarray(b_out, np.float32)
    in_maps_b = []
    for core in range(8):
        wsh = np.zeros((_H, _VP), np.float32)
        wsh[:, :_VS] = W_out[:, core * _VS:(core + 1) * _VS]
        bsh = np.zeros(_VP, np.float32)
        bsh[:_VS] = 1.5 * b_out[core * _VS:(core + 1) * _VS]
        W8f = wsh.astype(F8).astype(np.float32)
        wo8 = np.stack([_drpack(wsh), _drpack(16.0 * (wsh - W8f))])
        in_maps_b.append(dict(
            at8=at8,
            wo8=wo8,
            bo=np.ascontiguousarray(bsh.reshape(_VP // 128, 128).T),
        ))
    res_b = run_bass_kernel_spmd(proj_nc, in_maps_b, core_ids=list(range(8)),
                                 trace=trace)
    out = np.empty((_B, _T, _V), np.float32)
    for core in range(8):
        sh = res_b.results[core]["outT"][:_VS]          # [4000, 2048] bf16
        out[:, :, core * _VS:(core + 1) * _VS] = (
            sh.astype(np.float32).T.reshape(_B, _T, _VS))

    if trace:
        LAST_PERF["rec_ns"] = res_a.exec_time_ns
        LAST_PERF["proj_ns"] = res_b.exec_time_ns
    return out

